# revision 1
# baseline (speedup 1.0000x reference)
"""Trainium2 Bass kernel for DeepME edge-MLP (gnn_message_passing).

Contract: kernel(**inputs) takes FULL unsharded inputs (as produced by the
reference setup_inputs()) and returns the FULL [E, 1] float32 output.

Strategy: data-parallel over the edge dimension across 8 NeuronCores.
Embedding table and (small) MLP weights are replicated per core.

Per-core device program (SPMD, one Bass program):
  - edge indices / types resident in SBUF, partition-major layout
  - per 512-edge tile:
      gather src/dst embedding rows (indirect DMA, 128 rows per descriptor)
      PE-transpose into feature-on-partition layout
      branch matmuls (192->64 x3 on diff/diff^2/src*dst, 192->192 x2)
      relu+bias fused into PSUM eviction on the scalar engine
      PE-transpose down to edge-on-partition layout, LayerNorm via bn_stats
        (gamma/beta folded into the next layer's weights host-side)
      PE-transpose up, fusion MLP 576->192 (LN) ->192 ->3
      output probability = 1 / sum_j exp(l_j - l_{edge_type})
"""

import numpy as np

# ---------------------------------------------------------------------------
# problem constants (hardcoded per the harness contract)
E_TOTAL = 300000
N_NODES = 300000
H = 192
H3 = 64
NCORES = 8
P = 128
CH = 4                 # 128-edge chunks per tile
TILE = P * CH          # 512 edges per tile
E_PC = E_TOTAL // NCORES          # 37500 edges per core
NTILES = (E_PC + TILE - 1) // TILE  # 74
E_PAD = NTILES * TILE               # 37888
LN_EPS = 1e-5

_PROG_CACHE = {}


def _build_program(n_tiles, n_nodes, mmdt="f32", repeat=1):
    """Build the SPMD Bass program. Returns the Bass object."""
    from contextlib import ExitStack

    import concourse.bass as bass
    import concourse.bacc as bacc
    import concourse.tile as tile
    import concourse.mybir as mybir

    dt = mybir.dt
    f32 = dt.float32
    i32 = dt.int32
    MMDT = {"f32": f32, "f32r": dt.float32r, "bf16": dt.bfloat16,
            "f32rb": dt.float32r}[mmdt]
    EMBDT = dt.bfloat16 if mmdt in ("bf16", "f32rb") else f32
    TRDT = dt.bfloat16 if mmdt == "bf16" else f32   # transpose-path dtype
    GDT = EMBDT if EMBDT != f32 else TRDT           # gather-transpose dtype

    def rd(ap):
        # read view of an MMDT tile for elementwise engines
        return ap.bitcast(f32) if mmdt in ("f32r", "f32rb") else ap
    AF = mybir.ActivationFunctionType
    OP = mybir.AluOpType
    AX = mybir.AxisListType

    nedge_cols = n_tiles * CH

    nc = bacc.Bacc(trn_type="TRN2", target_bir_lowering=False, debug=False,
                   num_devices=NCORES)

    # ----- DRAM parameters ------------------------------------------------
    emb = nc.dram_tensor("emb", [n_nodes, H], EMBDT, kind="ExternalInput").ap()
    sidx_d = nc.dram_tensor("sidx", [P, nedge_cols], i32, kind="ExternalInput").ap()
    didx_d = nc.dram_tensor("didx", [P, nedge_cols], i32, kind="ExternalInput").ap()
    etf_d = nc.dram_tensor("etf", [P, nedge_cols], f32, kind="ExternalInput").ap()
    w1_d = nc.dram_tensor("w1", [H, H3], f32, kind="ExternalInput").ap()
    w2_d = nc.dram_tensor("w2", [H, H3], f32, kind="ExternalInput").ap()
    w3_d = nc.dram_tensor("w3", [H, H3], f32, kind="ExternalInput").ap()
    ws_d = nc.dram_tensor("ws", [H, H], f32, kind="ExternalInput").ap()
    wd_d = nc.dram_tensor("wd", [H, H], f32, kind="ExternalInput").ap()
    wf1_d = nc.dram_tensor("wf1", [3 * H, H], f32, kind="ExternalInput").ap()
    wf2_d = nc.dram_tensor("wf2", [H, H], f32, kind="ExternalInput").ap()
    wf3_d = nc.dram_tensor("wf3", [H, 4], f32, kind="ExternalInput").ap()
    nc1_d = nc.dram_tensor("nc1", [5, H], f32, kind="ExternalInput").ap()
    nc1f_d = nc.dram_tensor("nc1f", [1, H], f32, kind="ExternalInput").ap()
    # packed per-partition bias columns (see kernel() for layout)
    bias_d = nc.dram_tensor("biascol", [P, 12], f32, kind="ExternalInput").ap()
    # consts: identity(128x128) | iota3 (12) | c4N (20)
    cst_d = nc.dram_tensor("consts", [P, P + 12 + 20], f32, kind="ExternalInput").ap()
    out_d = nc.dram_tensor("out", [P, nedge_cols], f32, kind="ExternalOutput").ap()

    def mm(out, lhsT, rhs, start, stop=True):
        nc.tensor.matmul(out=out, lhsT=lhsT, rhs=rhs, start=start, stop=stop)

    with tile.TileContext(nc) as tc, ExitStack() as ctx:
        cpool = ctx.enter_context(tc.tile_pool(name="const", bufs=1))
        sb = ctx.enter_context(tc.tile_pool(name="work", bufs=1))
        sb2 = ctx.enter_context(tc.tile_pool(name="work2", bufs=2))
        pp = ctx.enter_context(tc.tile_pool(name="psum", bufs=1, space="PSUM"))

        # ----- resident tiles (loaded once) -------------------------------
        sidx = cpool.tile([P, nedge_cols], i32)
        didx = cpool.tile([P, nedge_cols], i32)
        etf = cpool.tile([P, nedge_cols], f32)
        outp = cpool.tile([P, nedge_cols], f32)
        nc.sync.dma_start(sidx[:], sidx_d[:])
        nc.sync.dma_start(didx[:], didx_d[:])
        nc.sync.dma_start(etf[:], etf_d[:])

        def wload(shape, src_ap, name):
            t_ = cpool.tile(shape, MMDT, name=name)
            if mmdt == "f32":
                nc.sync.dma_start(t_[:], src_ap)
            else:
                stg = cpool.tile(shape, f32, name=f"{name}_stg")
                nc.sync.dma_start(stg[:], src_ap)
                nc.vector.tensor_copy(t_[:], stg[:])
            return t_

        w1h = wload([P, H3], w1_d[0:P, :], "w1h")
        w1l = wload([H - P, H3], w1_d[P:H, :], "w1l")
        w2h = wload([P, H3], w2_d[0:P, :], "w2h")
        w2l = wload([H - P, H3], w2_d[P:H, :], "w2l")
        w3h = wload([P, H3], w3_d[0:P, :], "w3h")
        w3l = wload([H - P, H3], w3_d[P:H, :], "w3l")
        wsh = wload([P, H], ws_d[0:P, :], "wsh")
        wsl = wload([H - P, H], ws_d[P:H, :], "wsl")
        wdh = wload([P, H], wd_d[0:P, :], "wdh")
        wdl = wload([H - P, H], wd_d[P:H, :], "wdl")
        wf2h = wload([P, H], wf2_d[0:P, :], "wf2h")
        wf2l = wload([H - P, H], wf2_d[P:H, :], "wf2l")
        wf3h = wload([P, 4], wf3_d[0:P, :], "wf3h")
        wf3l = wload([H - P, 4], wf3_d[P:H, :], "wf3l")
        # fusion weight: five K-chunks (rows of [sx | dx | b1 | b2 | b3])
        wf1t = [wload([P, H], wf1_d[k * P:(k + 1) * P, :], f"wf1t{k}")
                for k in range(4)]
        wf1e = wload([64, H], wf1_d[512:576, :], "wf1e")
        nc1w = wload([5, H], nc1_d[:, :], "nc1w")
        nc1fw = wload([1, H], nc1f_d[:, :], "nc1fw")

        bias = cpool.tile([P, 12], f32)
        nc.sync.dma_start(bias[:], bias_d[:])
        cst = cpool.tile([P, P + 12 + 20], f32)
        nc.sync.dma_start(cst[:], cst_d[:])
        ident = cst[:, 0:P]
        identb = None
        if TRDT != f32 or EMBDT != f32:
            identbt = cpool.tile([P, P], dt.bfloat16, name="identbt")
            nc.vector.tensor_copy(identbt[:], cst[:, 0:P])
            identb = identbt[:]
        iota3 = cst[:, P:P + 12].rearrange("p (c t) -> p c t", t=3)
        c4n = cst[:, P + 12:P + 32].rearrange("p (c b) -> p c b", b=5)

        def tp(out, in_):
            k = in_.partition_size()
            idn = identb if (identb is not None
                             and in_.dtype == dt.bfloat16) else ident
            nc.tensor.transpose(out=out, in_=in_, identity=idn[0:k, 0:k])

        # bias column layout (see kernel()):
        # 0: [b1;b2]  1: [b3;0]  2: bs_hi  3: bs_lo  4: bd_hi  5: bd_lo
        # 6: bf1_hi   7: bf1_lo  8: bf2_hi 9: bf2_lo 10: [bf3;0] 11: 4*eps
        def bcol(j, np_=P):
            return bias[0:np_, j:j + 1]

        # ----- two-phase pipelined tile loop ------------------------------
        # part1(t): gather -> transposes -> branch matmuls -> relu evict ->
        #           transpose-down -> LN stats+finalize
        # part2(t): LN apply -> transpose-up -> fusion MLP -> softmax-select
        # Emitting part1(t) before part2(t-1) lets the PE work on tile t's
        # gathers/branches while the vector engines finish tile t-1's LN.
        # PSUM tags: part1 {A,B,H,C,F,G}, part2 {D,E}.

        def part1(t):
            srcG = sb2.tile([P, CH, H], EMBDT, tag="srcG", name="srcG")
            dstG = sb2.tile([P, CH, H], EMBDT, tag="dstG", name="dstG")
            for c in range(CH):
                nc.gpsimd.indirect_dma_start(
                    out=srcG[:, c, :], out_offset=None, in_=emb[:, :],
                    in_offset=bass.IndirectOffsetOnAxis(
                        ap=sidx[:, t * CH + c: t * CH + c + 1], axis=0))
                nc.gpsimd.indirect_dma_start(
                    out=dstG[:, c, :], out_offset=None, in_=emb[:, :],
                    in_offset=bass.IndirectOffsetOnAxis(
                        ap=didx[:, t * CH + c: t * CH + c + 1], axis=0))

            # transpose to feature-major: srcT = [192, 512] as two tiles
            sTA = pp.tile([P, TILE], GDT, tag="psA", name="sTA")
            dTA = pp.tile([P, TILE], GDT, tag="psH", name="dTA")
            sTB = pp.tile([64, TILE], GDT, tag="psB", name="sTB")
            srcTA = sb.tile([P, TILE], MMDT, tag="srcTA", bufs=2, name="srcTA")
            srcTB = sb.tile([64, TILE], MMDT, tag="srcTB", bufs=2, name="srcTB")
            dstTA = sb.tile([P, TILE], MMDT, tag="dstTA", bufs=2, name="dstTA")
            dstTB = sb.tile([64, TILE], MMDT, tag="dstTB", bufs=2, name="dstTB")
            for c in range(CH):
                cs = slice(c * P, (c + 1) * P)
                tp(sTA[:, cs], srcG[:, c, 0:P])
                tp(sTB[:, cs], srcG[:, c, P:H])
                tp(dTA[:, cs], dstG[:, c, 0:P])
            nc.any.tensor_copy(srcTB[:], sTB[:])
            dTB = pp.tile([64, TILE], GDT, tag="psB", name="dTB")
            for c in range(CH):
                cs = slice(c * P, (c + 1) * P)
                tp(dTB[:, cs], dstG[:, c, P:H])
            nc.any.tensor_copy(srcTA[:], sTA[:])
            nc.any.tensor_copy(dstTA[:], dTA[:])
            nc.any.tensor_copy(dstTB[:], dTB[:])

            # elementwise: diff, prod, diff^2 (feature-major)
            difA = sb.tile([P, TILE], MMDT, tag="difA", bufs=2, name="difA")
            difB = sb.tile([64, TILE], MMDT, tag="difB", bufs=2, name="difB")
            prdA = sb.tile([P, TILE], MMDT, tag="prdA", bufs=2, name="prdA")
            prdB = sb.tile([64, TILE], MMDT, tag="prdB", bufs=2, name="prdB")
            sqA = sb.tile([P, TILE], MMDT, tag="sqA", bufs=2, name="sqA")
            sqB = sb.tile([64, TILE], MMDT, tag="sqB", bufs=2, name="sqB")
            nc.vector.tensor_sub(difA[:], rd(srcTA[:]), rd(dstTA[:]))
            nc.vector.tensor_sub(difB[:], rd(srcTB[:]), rd(dstTB[:]))
            nc.vector.tensor_mul(prdA[:], rd(srcTA[:]), rd(dstTA[:]))
            nc.vector.tensor_mul(prdB[:], rd(srcTB[:]), rd(dstTB[:]))
            nc.scalar.activation(sqA[:], rd(difA[:]), AF.Square)
            nc.scalar.activation(sqB[:], rd(difB[:]), AF.Square)

            # branch matmuls; b1/b2/b3 sequentially share tag psC
            r_b = sb.tile([P, TILE], f32, tag="r_b", bufs=2, name="r_b")
            r_b3 = sb.tile([64, TILE], f32, tag="r_b3", bufs=2, name="r_b3")
            r_sxA = sb.tile([P, TILE], f32, tag="r_sxA", bufs=2, name="r_sxA")
            r_sxB = sb.tile([64, TILE], f32, tag="r_sxB", bufs=2, name="r_sxB")
            r_dxA = sb.tile([P, TILE], f32, tag="r_dxA", bufs=2, name="r_dxA")
            r_dxB = sb.tile([64, TILE], f32, tag="r_dxB", bufs=2, name="r_dxB")

            Pb1 = pp.tile([64, TILE], f32, tag="psC", name="Pb1")
            mm(Pb1[:, :], w1h[:], difA[:], start=True, stop=False)
            mm(Pb1[:, :], w1l[:], difB[:], start=False)
            nc.scalar.activation(r_b[0:64, :], Pb1[:], AF.Relu,
                                 bias=bias[0:64, 0:1])
            Pb2 = pp.tile([64, TILE], f32, tag="psC", name="Pb2")
            mm(Pb2[:, :], w2h[:], sqA[:], start=True, stop=False)
            mm(Pb2[:, :], w2l[:], sqB[:], start=False)
            nc.scalar.activation(r_b[64:128, :], Pb2[:], AF.Relu,
                                 bias=bias[64:128, 0:1])
            Pb3 = pp.tile([64, TILE], f32, tag="psC", name="Pb3")
            mm(Pb3[:, :], w3h[:], prdA[:], start=True, stop=False)
            mm(Pb3[:, :], w3l[:], prdB[:], start=False)
            nc.scalar.activation(r_b3[:], Pb3[:], AF.Relu, bias=bcol(1, 64))

            PsxA = pp.tile([P, TILE], f32, tag="psF", name="PsxA")
            PsxB = pp.tile([64, TILE], f32, tag="psG", name="PsxB")
            mm(PsxA[:, :], wsh[:, 0:P], srcTA[:], start=True, stop=False)
            mm(PsxA[:, :], wsl[:, 0:P], srcTB[:], start=False)
            mm(PsxB[:, :], wsh[:, P:H], srcTA[:], start=True, stop=False)
            mm(PsxB[:, :], wsl[:, P:H], srcTB[:], start=False)
            nc.scalar.activation(r_sxA[:], PsxA[:], AF.Relu, bias=bcol(2))
            nc.scalar.activation(r_sxB[:], PsxB[:], AF.Relu, bias=bcol(3, 64))
            PdxA = pp.tile([P, TILE], f32, tag="psF", name="PdxA")
            PdxB = pp.tile([64, TILE], f32, tag="psG", name="PdxB")
            mm(PdxA[:, :], wdh[:, 0:P], dstTA[:], start=True, stop=False)
            mm(PdxA[:, :], wdl[:, 0:P], dstTB[:], start=False)
            mm(PdxB[:, :], wdh[:, P:H], dstTA[:], start=True, stop=False)
            mm(PdxB[:, :], wdl[:, P:H], dstTB[:], start=False)
            nc.scalar.activation(r_dxA[:], PdxA[:], AF.Relu, bias=bcol(4))
            nc.scalar.activation(r_dxB[:], PdxB[:], AF.Relu, bias=bcol(5, 64))

            # transpose down to edge-major: r_e [128, CH, 576]
            # feature order: sx(192) dx(192) b1(64) b2(64) | b3(64)
            rTB = pp.tile([P, CH, 64], f32, tag="psB", name="rTB")
            r_e = sb.tile([P, CH, 576], f32, tag="r_e", bufs=2, name="r_e")
            dn_tags = ["psA", "psH", "psA", "psH"]
            for c in range(CH):
                cs = slice(c * P, (c + 1) * P)
                rTA = pp.tile([P, TILE], f32, tag=dn_tags[c], name=f"rTA{c}")
                tp(rTA[:, 0:P], r_sxA[:, cs])
                tp(rTA[:, P:192], r_sxB[:, cs])
                tp(rTA[:, 192:320], r_dxA[:, cs])
                tp(rTA[:, 320:384], r_dxB[:, cs])
                tp(rTA[:, 384:512], r_b[:, cs])
                tp(rTB[:, c, :], r_b3[:, cs])
                nc.any.tensor_copy(r_e[:, c, 0:512], rTA[:])
            nc.any.tensor_copy(r_e[:, :, 512:576], rTB[:])

            # LN stats: sum and sum-of-squares via tensor_reduce
            sq_e = sb.tile([P, CH, 576], f32, tag="sq_e", name="sq_e")
            nc.scalar.activation(sq_e[:], r_e[:], AF.Square)
            su = sb.tile([P, CH, 5], f32, tag="su", name="su")
            qu = sb.tile([P, CH, 5], f32, tag="qu", name="qu")
            r2v = r_e[:, :, 0:384].rearrange("p c (b f) -> p c b f", b=2)
            r3v = r_e[:, :, 384:576].rearrange("p c (b f) -> p c b f", b=3)
            q2v = sq_e[:, :, 0:384].rearrange("p c (b f) -> p c b f", b=2)
            q3v = sq_e[:, :, 384:576].rearrange("p c (b f) -> p c b f", b=3)
            nc.vector.tensor_reduce(out=su[:, :, 0:2], in_=r2v, axis=AX.X, op=OP.add)
            nc.vector.tensor_reduce(out=su[:, :, 2:5], in_=r3v, axis=AX.X, op=OP.add)
            nc.vector.tensor_reduce(out=qu[:, :, 0:2], in_=q2v, axis=AX.X, op=OP.add)
            nc.vector.tensor_reduce(out=qu[:, :, 2:5], in_=q3v, axis=AX.X, op=OP.add)
            # finalize: mu, istd (as is_t), q = mu*istd
            mu_t = sb.tile([P, CH, 5], f32, tag="mu_t", name="mu_t")
            ms_t = sb.tile([P, CH, 5], f32, tag="ms_t", name="ms_t")
            t_t = sb.tile([P, CH, 5], f32, tag="t_t", name="t_t")
            se_t = sb.tile([P, CH, 5], f32, tag="se_t", name="se_t")
            is_t = sb.tile([P, CH, 5], f32, tag="is_t", bufs=2, name="is_t")
            q_t = sb.tile([P, CH, 5], f32, tag="q_t", bufs=2, name="q_t")
            nc.vector.tensor_mul(mu_t[:], su[:], c4n[:, :, :])
            nc.vector.tensor_mul(ms_t[:], qu[:], c4n[:, :, :])
            nc.vector.scalar_tensor_tensor(
                out=t_t[:], in0=mu_t[:], scalar=1.0, in1=mu_t[:],
                op0=OP.mult, op1=OP.mult)
            nc.vector.tensor_sub(ms_t[:], ms_t[:], t_t[:])
            nc.scalar.activation(se_t[:], ms_t[:], AF.Ln, bias=bcol(11))
            nc.scalar.activation(is_t[:], se_t[:], AF.Exp, scale=-0.5)
            nc.vector.scalar_tensor_tensor(
                out=q_t[:], in0=mu_t[:], scalar=1.0, in1=is_t[:],
                op0=OP.mult, op1=OP.mult)
            qT_ps = pp.tile([5, TILE], f32, tag="psC", name="qT_ps")
            for c in range(CH):
                tp(qT_ps[:, c * P:(c + 1) * P], q_t[:, c, :])
            qrow = sb.tile([5, TILE], MMDT, tag="qrow", bufs=2, name="qrow")
            nc.any.tensor_copy(qrow[:], qT_ps[:])
            return r_e, is_t, qrow

        def part2(t, r_e, is_t, qrow):
            # apply scale only: y = r * IS ; the -mu*istd correction is folded
            # into the fusion matmul as a rank-5 term (lhsT = -colsum(Wf1_b))
            y_e = sb.tile([P, CH, 576], TRDT, tag="y_e", name="y_e")
            g1v = r_e[:, :, 0:384].rearrange("p c (b f) -> p c b f", b=2)
            g2v = r_e[:, :, 384:576].rearrange("p c (b f) -> p c b f", b=3)
            y1v = y_e[:, :, 0:384].rearrange("p c (b f) -> p c b f", b=2)
            y2v = y_e[:, :, 384:576].rearrange("p c (b f) -> p c b f", b=3)
            is2a = is_t[:, :, 0:2].unsqueeze(3).to_broadcast([P, CH, 2, H])
            is2b = is_t[:, :, 2:5].unsqueeze(3).to_broadcast([P, CH, 3, 64])
            nc.vector.tensor_mul(y1v, g1v, is2a)
            nc.vector.tensor_mul(y2v, g2v, is2b)

            # transpose up to feature-major y tiles (5 K-chunks of wf1)
            up_tags = ["psD", "psE", "psD", "psE"]
            yT = []
            for k in range(4):
                yT.append(pp.tile([P, TILE], TRDT, tag=up_tags[k], name=f"yT{k}"))
            yTE = pp.tile([64, TILE], TRDT, tag="psD", name="yTE")
            ysb = []
            for k in range(4):
                yk = sb.tile([P, TILE], MMDT, tag=f"ysb{k}", name=f"ysb{k}")
                ysb.append(yk)
            yke = sb.tile([64, TILE], MMDT, tag="ysbE", name="ysbE")
            for c in range(CH):
                cs = slice(c * P, (c + 1) * P)
                for k in range(4):
                    tp(yT[k][:, cs], y_e[:, c, k * P:(k + 1) * P])
            for k in range(4):
                nc.any.tensor_copy(ysb[k][:], yT[k][:])
            for c in range(CH):
                cs = slice(c * P, (c + 1) * P)
                tp(yTE[:, cs], y_e[:, c, 512:576])
            nc.any.tensor_copy(yke[:], yTE[:])

            # fusion matmul 576 -> 192
            zfA = pp.tile([P, TILE], f32, tag="psE", name="zfA")
            zfB = pp.tile([64, TILE], f32, tag="psD", name="zfB")
            for k in range(4):
                mm(zfA[:, :], wf1t[k][:, 0:P], ysb[k][:], start=(k == 0), stop=False)
            mm(zfA[:, :], wf1e[:, 0:P], yke[:], start=False, stop=False)
            mm(zfA[:, :], nc1w[:, 0:P], qrow[:], start=False)
            for k in range(4):
                mm(zfB[:, :], wf1t[k][:, P:H], ysb[k][:], start=(k == 0), stop=False)
            mm(zfB[:, :], wf1e[:, P:H], yke[:], start=False, stop=False)
            mm(zfB[:, :], nc1w[:, P:H], qrow[:], start=False)

            r_fA = sb.tile([P, TILE], f32, tag="r_fA", name="r_fA")
            r_fB = sb.tile([64, TILE], f32, tag="r_fB", name="r_fB")
            nc.scalar.activation(r_fA[:], zfA[:], AF.Relu, bias=bcol(6))
            nc.scalar.activation(r_fB[:], zfB[:], AF.Relu, bias=bcol(7, 64))

            # fusion LN (edge-major round trip)
            rfT01 = pp.tile([P, 2, H], f32, tag="psE", name="rfT01")
            rfT23 = pp.tile([P, 2, H], f32, tag="psD", name="rfT23")
            for c in range(CH):
                cs = slice(c * P, (c + 1) * P)
                dst = rfT01 if c < 2 else rfT23
                tp(dst[:, c % 2, 0:P], r_fA[:, cs])
                tp(dst[:, c % 2, P:H], r_fB[:, cs])
            rf_e = sb.tile([P, CH, H], f32, tag="rf_e", name="rf_e")
            nc.any.tensor_copy(rf_e[:, 0:2, :], rfT01[:])
            nc.any.tensor_copy(rf_e[:, 2:4, :], rfT23[:])

            sqf = sb.tile([P, CH, H], f32, tag="sqf", name="sqf")
            nc.scalar.activation(sqf[:], rf_e[:], AF.Square)
            suf = sb.tile([P, CH], f32, tag="suf", name="suf")
            quf = sb.tile([P, CH], f32, tag="quf", name="quf")
            nc.vector.tensor_reduce(out=suf[:], in_=rf_e[:], axis=AX.X, op=OP.add)
            nc.vector.tensor_reduce(out=quf[:], in_=sqf[:], axis=AX.X, op=OP.add)
            muf = sb.tile([P, CH], f32, tag="muf", name="muf")
            msf = sb.tile([P, CH], f32, tag="msf", name="msf")
            ttf = sb.tile([P, CH], f32, tag="ttf", name="ttf")
            sef = sb.tile([P, CH], f32, tag="sef", name="sef")
            is2f = sb.tile([P, CH], f32, tag="is2f", name="is2f")
            qf = sb.tile([P, CH], f32, tag="qf", name="qf")
            nc.vector.tensor_scalar(
                out=muf[:], in0=suf[:], scalar1=1.0 / H, scalar2=None, op0=OP.mult)
            nc.vector.tensor_scalar(
                out=msf[:], in0=quf[:], scalar1=1.0 / H, scalar2=None, op0=OP.mult)
            nc.vector.scalar_tensor_tensor(
                out=ttf[:], in0=muf[:], scalar=1.0, in1=muf[:],
                op0=OP.mult, op1=OP.mult)
            nc.vector.tensor_sub(msf[:], msf[:], ttf[:])
            nc.scalar.activation(sef[:], msf[:], AF.Ln, bias=bcol(11))
            nc.scalar.activation(is2f[:], sef[:], AF.Exp, scale=-0.5)
            nc.vector.scalar_tensor_tensor(
                out=qf[:], in0=muf[:], scalar=1.0, in1=is2f[:],
                op0=OP.mult, op1=OP.mult)
            qfT_ps = pp.tile([1, TILE], f32, tag="psD", name="qfT_ps")
            for c in range(CH):
                tp(qfT_ps[:, c * P:(c + 1) * P], qf[:, c:c + 1])
            qfrow = sb.tile([1, TILE], MMDT, tag="qfrow", name="qfrow")
            nc.any.tensor_copy(qfrow[:], qfT_ps[:])

            yf_e = sb.tile([P, CH, H], TRDT, tag="yf_e", name="yf_e")
            is2fb = is2f[:].unsqueeze(2).to_broadcast([P, CH, H])
            nc.vector.tensor_mul(yf_e[:], rf_e[:], is2fb)

            yfTA = pp.tile([P, TILE], TRDT, tag="psE", name="yfTA")
            yfTB = pp.tile([64, TILE], TRDT, tag="psD", name="yfTB")
            for c in range(CH):
                cs = slice(c * P, (c + 1) * P)
                tp(yfTA[:, cs], yf_e[:, c, 0:P])
                tp(yfTB[:, cs], yf_e[:, c, P:H])
            yfA = sb.tile([P, TILE], MMDT, tag="yfA", name="yfA")
            yfB = sb.tile([64, TILE], MMDT, tag="yfB", name="yfB")
            nc.any.tensor_copy(yfA[:], yfTA[:])
            nc.any.tensor_copy(yfB[:], yfTB[:])

            # fc2: 192 -> 192, relu
            z2A = pp.tile([P, TILE], f32, tag="psE", name="z2A")
            z2B = pp.tile([64, TILE], f32, tag="psD", name="z2B")
            mm(z2A[:, :], wf2h[:, 0:P], yfA[:], start=True, stop=False)
            mm(z2A[:, :], wf2l[:, 0:P], yfB[:], start=False, stop=False)
            mm(z2A[:, :], nc1fw[:, 0:P], qfrow[:], start=False)
            mm(z2B[:, :], wf2h[:, P:H], yfA[:], start=True, stop=False)
            mm(z2B[:, :], wf2l[:, P:H], yfB[:], start=False, stop=False)
            mm(z2B[:, :], nc1fw[:, P:H], qfrow[:], start=False)
            r2A = sb.tile([P, TILE], MMDT, tag="r2A", name="r2A")
            r2B = sb.tile([64, TILE], MMDT, tag="r2B", name="r2B")
            nc.scalar.activation(r2A[:], z2A[:], AF.Relu, bias=bcol(8))
            nc.scalar.activation(r2B[:], z2B[:], AF.Relu, bias=bcol(9, 64))

            # fc3: 192 -> 3 logits (padded to 4)
            zl = pp.tile([4, TILE], f32, tag="psE", name="zl")
            mm(zl[:, :], wf3h[:], r2A[:], start=True, stop=False)
            mm(zl[:, :], wf3l[:], r2B[:], start=False)
            l_sb = sb.tile([3, TILE], f32, tag="l_sb", name="l_sb")
            nc.scalar.activation(l_sb[:], zl[0:3, :], AF.Copy)
            nc.vector.tensor_scalar(
                out=l_sb[:], in0=l_sb[:], scalar1=bcol(10, 3), scalar2=None,
                op0=OP.add)

            # transpose logits to edge-major [128, CH, 3]
            lT = pp.tile([P, CH, 3], f32, tag="psD", name="lT")
            for c in range(CH):
                tp(lT[:, c, :], l_sb[:, c * P:(c + 1) * P])
            l_e = sb.tile([P, CH, 3], f32, tag="l_e", name="l_e")
            nc.any.tensor_copy(l_e[:], lT[:])

            # p = 1 / sum_j exp(l_j - l_sel)   (gpsimd for the small TTs)
            etb = etf[:, t * CH:(t + 1) * CH].unsqueeze(2).to_broadcast([P, CH, 3])
            oh = sb.tile([P, CH, 3], f32, tag="oh", name="oh")
            nc.vector.tensor_tensor(out=oh[:], in0=etb, in1=iota3,
                                    op=OP.is_equal)
            nc.vector.tensor_mul(oh[:], oh[:], l_e[:])
            sel = sb.tile([P, CH], f32, tag="sel", name="sel")
            nc.vector.tensor_reduce(out=sel[:], in_=oh[:], axis=AX.X, op=OP.add)
            selb = sel[:].unsqueeze(2).to_broadcast([P, CH, 3])
            nc.vector.tensor_sub(l_e[:], l_e[:], selb)
            ex = sb.tile([P, CH, 3], f32, tag="ex", name="ex")
            nc.scalar.activation(ex[:], l_e[:], AF.Exp)
            den = sb.tile([P, CH], f32, tag="den", name="den")
            nc.vector.tensor_reduce(out=den[:], in_=ex[:], axis=AX.X, op=OP.add)
            nc.vector.reciprocal(outp[:, t * CH:(t + 1) * CH], den[:])

        def whole_body(_iv=None):
            prev = None
            for t in range(n_tiles):
                cur = part1(t)
                if prev is not None:
                    part2(t - 1, *prev)
                prev = cur
            part2(n_tiles - 1, *prev)

        if repeat > 1:
            with tc.For_i(0, repeat, 1):
                whole_body()
        else:
            whole_body()

        # write all outputs
        nc.sync.dma_start(out_d[:], outp[:])

    # Pin the ACT table set: keep only natural_log_exp_and_others (covers
    # Relu/Square/Ln/Exp/Copy/Identity) so the table-load pass never cycles
    # sets. Indices must stay aligned with act_info.json, so empty the other
    # sets rather than removing them.
    import concourse.bacc as _bacc_mod
    _orig_gat = _bacc_mod.get_activation_tables

    def _pinned_tables(arch):
        tabs = _orig_gat(arch)
        return {name: (s if name == "natural_log_exp_and_others" else set())
                for name, s in tabs.items()}

    _bacc_mod.get_activation_tables = _pinned_tables
    try:
        nc.compile()
    finally:
        _bacc_mod.get_activation_tables = _orig_gat
    return nc


def _get_program(n_tiles=NTILES, n_nodes=N_NODES, mmdt="f32", repeat=1):
    key = (n_tiles, n_nodes, mmdt, repeat)
    if key not in _PROG_CACHE:
        _PROG_CACHE[key] = _build_program(n_tiles, n_nodes, mmdt, repeat)
    return _PROG_CACHE[key]


_EDGE_PERM = {"perm": None}


def _host_prep(inputs, n_tiles=NTILES, n_cores=NCORES, e_pc=E_PC, mmdt="f32"):
    """Fold LN gains/betas into fusion weights; build per-core input maps."""
    f = lambda k: np.asarray(inputs[k], np.float32)
    kge = f("kge_emb")
    ei = np.asarray(inputs["edge_index"]).astype(np.int64)
    et = np.asarray(inputs["edge_type"]).astype(np.int64)
    # sort edges by src node id: each core's src gathers then walk a
    # contiguous ~N/8 slice of the table almost sequentially (better DRAM
    # locality); dst stays random. Output is inverse-permuted in _unshard.
    perm = np.argsort(ei[0], kind="stable")
    _EDGE_PERM["perm"] = perm
    ei = ei[:, perm]
    et = et[perm]
    W1, b1, g1, be1 = f("W1"), f("b1"), f("g1"), f("be1")
    W2, b2, g2, be2 = f("W2"), f("b2"), f("g2"), f("be2")
    W3, b3, g3, be3 = f("W3"), f("b3"), f("g3"), f("be3")
    Ws, bs, gs, bes = f("Ws"), f("bs"), f("gs"), f("bes")
    Wd, bd, gd, bed = f("Wd"), f("bd"), f("gd"), f("bed")
    Wf1, bf1, gf, bef = f("Wf1"), f("bf1"), f("gf"), f("bef")
    Wf2, bf2 = f("Wf2"), f("bf2")
    Wf3, bf3 = f("Wf3"), f("bf3")

    # device concat order == reference concat order: [sx, dx, b1, b2, b3]
    g_cat = np.concatenate([gs, gd, g1, g2, g3])
    be_cat = np.concatenate([bes, bed, be1, be2, be3])
    Wf1_eff = g_cat[:, None] * Wf1
    bf1_eff = bf1 + be_cat @ Wf1
    Wf2_eff = gf[:, None] * Wf2
    bf2_eff = bf2 + bef @ Wf2

    # bias columns [128, 11]
    bias_mat = np.zeros((P, 12), np.float32)
    bias_mat[:, 11] = LN_EPS
    bias_mat[0:64, 0] = b1; bias_mat[64:128, 0] = b2
    bias_mat[0:64, 1] = b3
    bias_mat[:, 2] = bs[0:P]; bias_mat[0:64, 3] = bs[P:H]
    bias_mat[:, 4] = bd[0:P]; bias_mat[0:64, 5] = bd[P:H]
    bias_mat[:, 6] = bf1_eff[0:P]; bias_mat[0:64, 7] = bf1_eff[P:H]
    bias_mat[:, 8] = bf2_eff[0:P]; bias_mat[0:64, 9] = bf2_eff[P:H]
    bias_mat[0:3, 10] = bf3

    # consts [128, 128+12+20]: identity | iota3 | c4N
    cst = np.zeros((P, P + 12 + 20), np.float32)
    cst[:, 0:P] = np.eye(P, dtype=np.float32)
    cst[:, P:P + 12] = np.tile(np.arange(3, dtype=np.float32), CH)[None, :]
    c4n = np.array([1.0 / H, 1.0 / H, 1.0 / 64, 1.0 / 64, 1.0 / 64],
                   np.float32)
    cst[:, P + 12:P + 32] = np.tile(c4n, CH)[None, :]

    e_pad = n_tiles * TILE

    def arrange(a):
        buf = np.zeros(e_pad, a.dtype)
        buf[:e_pc] = a
        return np.ascontiguousarray(
            buf.reshape(n_tiles, CH, P).transpose(2, 0, 1).reshape(P, -1))

    if mmdt in ("bf16", "f32rb"):
        import ml_dtypes
        kge = kge.astype(ml_dtypes.bfloat16)
    nc1 = np.zeros((5, H), np.float32)
    for b, (lo, hi) in enumerate(((0, 192), (192, 384), (384, 448),
                                  (448, 512), (512, 576))):
        nc1[b] = -Wf1_eff[lo:hi].sum(axis=0)
    nc1f = -Wf2_eff.sum(axis=0, keepdims=True)
    Wf3p = np.zeros((H, 4), np.float32)
    Wf3p[:, 0:3] = Wf3
    shared = dict(emb=kge, w1=W1, w2=W2, w3=W3, ws=Ws, wd=Wd,
                  wf1=Wf1_eff, wf2=Wf2_eff, wf3=Wf3p,
                  nc1=nc1, nc1f=nc1f,
                  biascol=bias_mat, consts=cst)
    in_maps = []
    for core in range(n_cores):
        lo = core * e_pc
        m = dict(shared)
        m["sidx"] = arrange(ei[0, lo:lo + e_pc].astype(np.int32))
        m["didx"] = arrange(ei[1, lo:lo + e_pc].astype(np.int32))
        m["etf"] = arrange(et[lo:lo + e_pc].astype(np.float32))
        in_maps.append(m)
    return in_maps


def _unshard(results, n_tiles=NTILES, n_cores=NCORES, e_pc=E_PC):
    outs = []
    for core in range(n_cores):
        o = np.asarray(results[core]["out"], np.float32)
        o = o.reshape(P, n_tiles, CH).transpose(1, 2, 0).reshape(-1)[:e_pc]
        outs.append(o)
    cat = np.concatenate(outs)
    perm = _EDGE_PERM["perm"]
    if perm is not None:
        inv = np.empty_like(cat)
        inv[perm] = cat
        cat = inv
    return cat[:, None].astype(np.float32)


MMDT_MODE = "f32r"


def kernel(**inputs):
    from concourse.bass_utils import run_bass_kernel_spmd
    nc = _get_program(mmdt=MMDT_MODE)
    in_maps = _host_prep(inputs, mmdt=MMDT_MODE)
    res = run_bass_kernel_spmd(nc, in_maps, list(range(NCORES)))
    return _unshard(res.results)



# revision 55
# speedup vs baseline: 3.3160x; 3.3160x over previous
"""Trainium2 Bass kernel for DeepME edge-MLP (gnn_message_passing).

Contract: kernel(**inputs) takes FULL unsharded inputs (as produced by the
reference setup_inputs()) and returns the FULL [E, 1] float32 output.

Strategy: data-parallel over the edge dimension across 8 NeuronCores.
Embedding table and (small) MLP weights are replicated per core.

v3 device program — feature-major, bf16 compute, f32 PSUM accumulation:
  per 512-edge tile:
    one multi-column indirect DMA per side gathers 512 bf16 embedding rows
    PE-transpose to feature-major; B-halves (features 128:192) live in
      persistent [65, 512] tiles whose row 64 is a constant 1.0 so that
      layer biases ride as an extra weight row (no bias in evictions)
    branch matmuls into packed PSUM groups; 3 pure-relu evictions
      produce h [128, 5, 512] (slices: b1|b2, b3|sx_lo, sx_hi, dx_hi,
      dx_lo|0)
    LN statistics via 1/n-mask matmuls (partition sums); var = E[h^2]-mu^2
      with the mu^2 term subtracted through a -I matmul; istd broadcast to
      [128,*,512] maps by block-mask matmuls; y = h * map on the DVE
    fusion matmul consumes y slices; bias and the rank-5 -mu*istd
      correction ride on an aux [6, 512] rhs (ones row + q rows)
    fusion LN same scheme; fc2 with aux [2, 512] (ones + qf); fc3 -> 3
      logits (+bf3 via a const ones-row rhs)
    logits are DMA'd out; the 3-way softmax-select epilogue runs in
      host numpy inside kernel()
"""

import numpy as np

# ---------------------------------------------------------------------------
# problem constants (hardcoded per the harness contract)
E_TOTAL = 300000
N_NODES = 300000
H = 192
H3 = 64
NCORES = 8
P = 128
CH = 4                 # 128-edge chunks per tile
TILE = P * CH          # 512 edges per tile
E_PC = E_TOTAL // NCORES          # 37500 edges per core
NTILES = (E_PC + TILE - 1) // TILE  # 74
E_PAD = NTILES * TILE               # 37888
LN_EPS = 1e-5

_PROG_CACHE = {}


def _build_program(n_tiles, n_nodes, mmdt="bf16", repeat=1):
    """Build the SPMD Bass program. Returns the Bass object."""
    from contextlib import ExitStack

    import concourse.bass as bass
    import concourse.bacc as bacc
    import concourse.tile as tile
    import concourse.mybir as mybir

    dt = mybir.dt
    f32 = dt.float32
    i32 = dt.int32
    bf16 = dt.bfloat16
    assert mmdt == "bf16"

    f32r = dt.float32r

    def rd(ap):
        return ap.bitcast(f32) if ap.dtype == dt.float32r else ap
    AF = mybir.ActivationFunctionType
    OP = mybir.AluOpType

    ncol_idx = n_tiles * CH          # sidx/didx columns
    ncol_l = n_tiles * TILE          # logits columns ([4, T*512])

    nc = bacc.Bacc(trn_type="TRN2", target_bir_lowering=False, debug=False,
                   num_devices=NCORES)

    # ----- DRAM parameters (weights shipped pre-packed in bf16) -----------
    def din(name, shape, dtype=bf16):
        return nc.dram_tensor(name, shape, dtype, kind="ExternalInput").ap()

    emb = din("emb", [n_nodes, H])
    sidx_d = din("sidx", [P, ncol_idx], i32)
    didx_d = din("didx", [P, ncol_idx], i32)
    # branch weights: *h = K-rows 0:128, *l = K-rows 128:192 + bias row;
    # PS1 contributors are zero-padded to full 128-col outputs so no mm
    # writes at a nonzero base partition
    w1h_d = din("w1h", [P, P], f32r); w1l_d = din("w1l", [65, P], f32r)
    w2h_d = din("w2h", [P, P], f32r); w2l_d = din("w2l", [65, P], f32r)
    w3h_d = din("w3h", [P, P], f32r); w3l_d = din("w3l", [65, P], f32r)
    wxh_d = din("wxh", [P, P], f32r); wxl_d = din("wxl", [65, P], f32r)
    wsh_d = din("wsh", [P, P], f32r); wsl_d = din("wsl", [65, P], f32r)
    wdh_d = din("wdh", [P, 2, P], f32r); wdl_d = din("wdl", [65, 2, P], f32r)
    # fusion: per y-slice lhsT chunks for out slice0 (feat 0:128) and
    # slice1 (feat 128:192 zero-extended), plus aux [6, 128] x2
    wf1A_d = din("wf1A", [P, 5, P], f32r)
    wf1B_d = din("wf1B", [P, 5, P], f32r)
    wf1xA_d = din("wf1xA", [6, P], f32r)
    wf1xB_d = din("wf1xB", [6, P], f32r)
    wf2A_d = din("wf2A", [P, 2, P], f32r)
    wf2B_d = din("wf2B", [P, 2, P], f32r)
    wf2xA_d = din("wf2xA", [2, P], f32r)
    wf2xB_d = din("wf2xB", [2, P], f32r)
    wf3_d = din("wf3", [P, 2, 4], f32r)
    wf3x_d = din("wf3x", [1, 4], f32r)
    # masks: stat masks [128,5,5] + [1x...] packed, bcast masks, -I5
    smu_d = din("smu", [P, 5 * 5], f32r)  # per-h-slice E[h] stat masks
    smq_d = din("smq", [P, 5 * 5], f32r)  # per-h-slice E[h^2] stat masks
    smf_d = din("smf", [P, 2], f32r)      # f-LN E[h] stat masks
    smqf_d = din("smqf", [P, 2], f32r)    # f-LN E[h^2] stat masks
    bcm_d = din("bcm", [5, 5 * P], f32r)  # istd bcast masks per slice
    bcf_d = din("bcf", [1, 2 * P], f32r)  # f-LN bcast masks
    neg5_d = din("neg5", [5, 5], f32r)    # -I5
    onesr_d = din("onesr", [1, TILE], f32r)  # const ones row (bf3 rhs)
    bias_d = din("biascol", [P, 4], f32)  # LN bias columns
    cst_d = din("consts", [P, P])         # bf16 identity
    out_d = nc.dram_tensor("out", [4, ncol_l], f32, kind="ExternalOutput").ap()

    def mm(out, lhsT, rhs, start, stop=True):
        nc.tensor.matmul(out=out, lhsT=lhsT, rhs=rhs, start=start, stop=stop)

    with tile.TileContext(nc) as tc, ExitStack() as ctx:
        cpool = ctx.enter_context(tc.tile_pool(name="const", bufs=1))
        sb = ctx.enter_context(tc.tile_pool(name="work", bufs=1))
        sb2 = ctx.enter_context(tc.tile_pool(name="work2", bufs=2))
        pp = ctx.enter_context(tc.tile_pool(name="psum", bufs=1, space="PSUM"))

        # ----- resident tiles (loaded once) -------------------------------
        def cload(shape, src_ap, name, dtype=bf16):
            t_ = cpool.tile(shape, dtype, name=name)
            nc.sync.dma_start(t_[:], src_ap)
            return t_

        sidx = cload([P, ncol_idx], sidx_d[:], "sidx", i32)
        didx = cload([P, ncol_idx], didx_d[:], "didx", i32)
        w1h = cload([P, P], w1h_d[:], "w1h", f32r)
        w1l = cload([65, P], w1l_d[:], "w1l", f32r)
        w2h = cload([P, P], w2h_d[:], "w2h", f32r)
        w2l = cload([65, P], w2l_d[:], "w2l", f32r)
        w3h = cload([P, P], w3h_d[:], "w3h", f32r)
        w3l = cload([65, P], w3l_d[:], "w3l", f32r)
        wxh = cload([P, P], wxh_d[:], "wxh", f32r)
        wxl = cload([65, P], wxl_d[:], "wxl", f32r)
        wsh = cload([P, P], wsh_d[:], "wsh", f32r)
        wsl = cload([65, P], wsl_d[:], "wsl", f32r)
        wdh = cload([P, 2, P], wdh_d[:], "wdh", f32r)
        wdl = cload([65, 2, P], wdl_d[:], "wdl", f32r)
        wf1A = cload([P, 5, P], wf1A_d[:], "wf1A", f32r)
        wf1B = cload([P, 5, P], wf1B_d[:], "wf1B", f32r)
        wf1xA = cload([6, P], wf1xA_d[:], "wf1xA", f32r)
        wf1xB = cload([6, P], wf1xB_d[:], "wf1xB", f32r)
        wf2A = cload([P, 2, P], wf2A_d[:], "wf2A", f32r)
        wf2B = cload([P, 2, P], wf2B_d[:], "wf2B", f32r)
        wf2xA = cload([2, P], wf2xA_d[:], "wf2xA", f32r)
        wf2xB = cload([2, P], wf2xB_d[:], "wf2xB", f32r)
        wf3 = cload([P, 2, 4], wf3_d[:], "wf3", f32r)
        wf3x = cload([1, 4], wf3x_d[:], "wf3x", f32r)
        smu_t = cload([P, 25], smu_d[:], "smu", f32r)
        smu = smu_t[:].rearrange("p (s k) -> p s k", k=5)       # [128,5,5]
        smq_t = cload([P, 25], smq_d[:], "smq", f32r)
        smq = smq_t[:].rearrange("p (s k) -> p s k", k=5)       # [128,5,5]
        smf = cload([P, 2], smf_d[:], "smf", f32r)
        smqf = cload([P, 2], smqf_d[:], "smqf", f32r)
        bcm_t = cload([5, 5 * P], bcm_d[:], "bcm", f32r)
        bcm = bcm_t[:].rearrange("p (s m) -> p s m", m=P)       # [5,5,128]
        bcf_t = cload([1, 2 * P], bcf_d[:], "bcf", f32r)
        bcf = bcf_t[:].rearrange("p (s m) -> p s m", m=P)       # [1,2,128]
        neg5 = cload([5, 5], neg5_d[:], "neg5", f32r)
        onesr = cload([1, TILE], onesr_d[:], "onesr", f32r)
        bias = cload([P, 4], bias_d[:], "bias", f32)
        cst = cload([P, P], cst_d[:], "ident")
        ident = cst[:]

        def tp(out, in_):
            k = in_.partition_size()
            nc.tensor.transpose(out=out, in_=in_, identity=ident[0:k, 0:k])

        # bias cols: 0 = k*eps rows 0:5, 1 = ln(c*n) rows 0:5,
        #            2 = kf*eps row 0,   3 = ln(cf*n) row 0
        def bcol(j, np_=P):
            return bias[0:np_, j:j + 1]

        # persistent [65, 512] operand tiles: row 64 is a constant 1.0 so
        # bias rows in the *l weights apply; writers only touch rows 0:64
        def ones_row_tile(name):
            t_ = cpool.tile([65, TILE], f32r, name=name)
            nc.vector.memset(rd(t_[64:65, :]), 1.0)
            return t_

        srcTB = ones_row_tile("srcTB")
        dstTB = ones_row_tile("dstTB")
        difB = ones_row_tile("difB")
        prdB = ones_row_tile("prdB")
        sqB = ones_row_tile("sqB")
        # aux fusion rhs: rows 0:5 = q (written per tile), row 5 = 1
        yx0 = cpool.tile([6, TILE], f32r, name="yx0")
        yx1 = cpool.tile([6, TILE], f32r, name="yx1")
        yx2 = cpool.tile([6, TILE], f32r, name="yx2")
        nc.vector.memset(rd(yx0[:, :]), 1.0)
        nc.vector.memset(rd(yx1[:, :]), 1.0)
        nc.vector.memset(rd(yx2[:, :]), 1.0)
        yxs = [yx0, yx1, yx2]
        # aux fc2 rhs: row0 = qf, row1 = 1
        yfx0 = cpool.tile([2, TILE], f32r, name="yfx0")
        yfx1 = cpool.tile([2, TILE], f32r, name="yfx1")
        nc.vector.memset(rd(yfx0[:, :]), 1.0)
        nc.vector.memset(rd(yfx1[:, :]), 1.0)
        yfxs = [yfx0, yfx1]

        # ----- five-phase pipelined tile loop -----------------------------
        # A(t): gather, transpose, elementwise, branch matmuls, evictions,
        #       squares
        # B(t): stat matmuls, istd, bcast maps, y = h*map, q rows
        # C(t): fusion matmuls, eviction, squares
        # D(t): f-LN stats, istd_f, map, yf, qf
        # E(t): fc2, eviction, fc3 logits, copy out, DMA
        # emission: A(t) C(t-1) B(t) D(t-1) E(t-1)

        def phaseG(t):
            srcG = sb2.tile([P, CH, H], bf16, tag="srcG", name="srcG")
            dstG = sb2.tile([P, CH, H], bf16, tag="dstG", name="dstG")
            for c in range(CH):
                cc = slice(t * CH + c, t * CH + c + 1)
                nc.gpsimd.indirect_dma_start(
                    out=srcG[:, c, :], out_offset=None, in_=emb[:, :],
                    in_offset=bass.IndirectOffsetOnAxis(ap=sidx[:, cc],
                                                        axis=0))
                nc.gpsimd.indirect_dma_start(
                    out=dstG[:, c, :], out_offset=None, in_=emb[:, :],
                    in_offset=bass.IndirectOffsetOnAxis(ap=didx[:, cc],
                                                        axis=0))
            return srcG, dstG

        def phaseA(t, srcG, dstG):
            # transposes through a 2KB psum tag, src then dst
            sTs = pp.tile([P, 2, TILE], bf16, tag="pT", name="sTs")
            for c in range(CH):
                cs = slice(c * P, (c + 1) * P)
                tp(sTs[:, 0, cs], srcG[:, c, 0:P])
                tp(sTs[0:64, 1, cs], srcG[:, c, P:H])
            srcTA = sb.tile([P, TILE], f32r, tag="srcTA", name="srcTA")
            dstTA = sb.tile([P, TILE], f32r, tag="dstTA", name="dstTA")
            nc.any.tensor_copy(rd(srcTA[:]), sTs[:, 0, :])
            nc.any.tensor_copy(rd(srcTB[0:64, :]), sTs[0:64, 1, :])
            sTd = pp.tile([P, 2, TILE], bf16, tag="pT", name="sTd")
            for c in range(CH):
                cs = slice(c * P, (c + 1) * P)
                tp(sTd[:, 0, cs], dstG[:, c, 0:P])
                tp(sTd[0:64, 1, cs], dstG[:, c, P:H])
            nc.any.tensor_copy(rd(dstTA[:]), sTd[:, 0, :])
            nc.any.tensor_copy(rd(dstTB[0:64, :]), sTd[0:64, 1, :])

            # elementwise: diff, prod, diff^2 (A on DVE, B on gpsimd)
            difA = sb.tile([P, TILE], f32r, tag="difA", name="difA")
            prdA = sb.tile([P, TILE], f32r, tag="prdA", name="prdA")
            sqA = sb.tile([P, TILE], f32r, tag="sqA", name="sqA")
            nc.vector.tensor_sub(rd(difA[:]), rd(srcTA[:]), rd(dstTA[:]))
            nc.vector.tensor_mul(rd(prdA[:]), rd(srcTA[:]), rd(dstTA[:]))
            nc.scalar.activation(rd(sqA[:]), rd(difA[:]), AF.Square)
            nc.gpsimd.tensor_sub(rd(difB[0:64, :]), rd(srcTB[0:64, :]),
                                 rd(dstTB[0:64, :]))
            nc.gpsimd.tensor_mul(rd(prdB[0:64, :]), rd(srcTB[0:64, :]),
                                 rd(dstTB[0:64, :]))
            nc.gpsimd.tensor_mul(rd(sqB[0:64, :]), rd(difB[0:64, :]),
                                 rd(difB[0:64, :]))

            # branch matmuls into packed PSUM groups (biases ride on the
            # ones row of the B operands)
            # PS1 [128,2,512]: s0 = b1(0:64) + b2(64:128); s1 = b3 + sx_lo
            # PS2 [128,2,512]: s0 = sx_hi;  s1 = dx_hi
            # PS3 [128,512]:   dx_lo zero-extended
            PS1 = pp.tile([P, 2, TILE], f32, tag="pA", name="PS1")
            mm(PS1[:, 0, :], w1h[:], difA[:], start=True, stop=False)
            mm(PS1[:, 0, :], w1l[:], difB[:], start=False, stop=False)
            mm(PS1[:, 0, :], w2h[:], sqA[:], start=False, stop=False)
            mm(PS1[:, 0, :], w2l[:], sqB[:], start=False)
            mm(PS1[:, 1, :], w3h[:], prdA[:], start=True, stop=False)
            mm(PS1[:, 1, :], w3l[:], prdB[:], start=False, stop=False)
            mm(PS1[:, 1, :], wxh[:], srcTA[:], start=False, stop=False)
            mm(PS1[:, 1, :], wxl[:], srcTB[:], start=False)
            PS2 = pp.tile([P, 2, TILE], f32, tag="pB", name="PS2")
            mm(PS2[:, 0, :], wsh[:], srcTA[:], start=True, stop=False)
            mm(PS2[:, 0, :], wsl[:], srcTB[:], start=False)
            mm(PS2[:, 1, :], wdh[:, 0, :], dstTA[:], start=True, stop=False)
            mm(PS2[:, 1, :], wdl[:, 0, :], dstTB[:], start=False)
            PS3 = pp.tile([P, TILE], f32, tag="pF", name="PS3")
            mm(PS3[:, :], wdh[:, 1, :], dstTA[:], start=True, stop=False)
            mm(PS3[:, :], wdl[:, 1, :], dstTB[:], start=False)

            # pure-relu evictions into h [128, 5, 512]
            # h slices: 0 = b1|b2, 1 = b3|sx_lo, 2 = sx_hi, 3 = dx_hi,
            #           4 = dx_lo|zeros
            h_p = sb.tile([P, 5, TILE], f32r, tag="h_p", bufs=3, name="h_p")
            hs_p = sb.tile([P, 5, TILE], f32r, tag="hs_p", bufs=3, name="hs_p")
            nc.scalar.activation(rd(h_p[:, 0:2, :]), PS1[:], AF.Relu)
            nc.vector.tensor_mul(rd(hs_p[:, 0:2, :]), rd(h_p[:, 0:2, :]),
                                 rd(h_p[:, 0:2, :]))
            nc.scalar.activation(rd(h_p[:, 2:4, :]), PS2[:], AF.Relu)
            nc.scalar.activation(rd(h_p[:, 4, :]), PS3[:], AF.Relu)
            nc.gpsimd.tensor_mul(rd(hs_p[:, 2:4, :]), rd(h_p[:, 2:4, :]),
                                 rd(h_p[:, 2:4, :]))
            nc.gpsimd.tensor_mul(rd(hs_p[:, 4, :]), rd(h_p[:, 4, :]),
                                 rd(h_p[:, 4, :]))
            return h_p, hs_p

        def phaseB1(t, h_p, hs_p):
            # stat matmuls; masks carry 1/n -> SU = mu, SQ = var (after the
            # -I5 * mu^2 accumulation); SUQ packed [10, 512]
            SUQ = pp.tile([37, TILE], f32, tag="pS", name="SUQ")
            for s in range(5):
                mm(SUQ[0:5, :], smu[:, s, :], h_p[:, s, :],
                   start=(s == 0), stop=(s == 4))
            mur2 = sb.tile([5, TILE], f32r, tag="mur2", name="mur2")
            nc.scalar.activation(rd(mur2[:]), SUQ[0:5, :], AF.Square)
            for s in range(5):
                mm(SUQ[32:37, :], smq[:, s, :], hs_p[:, s, :],
                   start=(s == 0), stop=False)
            mm(SUQ[32:37, :], neg5[:], mur2[:], start=False)

            # istd rows: is = exp(-0.5 ln(var + eps)); q = mu * is
            se = sb.tile([5, TILE], f32, tag="se", name="se")
            is_ = sb.tile([5, TILE], f32r, tag="is_", bufs=2, name="is_")
            nc.scalar.activation(se[:], SQ[:, :], AF.Ln, bias=bcol(0, 5))
            nc.scalar.activation(rd(is_[:]), se[:], AF.Exp, scale=-0.5,
                                 bias=bcol(1, 5))
            yx = yxs[t % 3]
            nc.vector.scalar_tensor_tensor(
                out=rd(yx[0:5, :]), in0=SUQ[0:5, :], scalar=1.0,
                in1=rd(is_[:]),
                op0=OP.mult, op1=OP.mult)
            return (is_,)

        def phaseB2(t, h_p, hs_p, is_):
            # istd maps + y = h * map; map groups through psum tags
            y_p = sb.tile([P, 5, TILE], f32r, tag="y_p", bufs=2, name="y_p")
            MG1 = pp.tile([P, 2, TILE], f32, tag="pA", name="MG1")
            mm(MG1[:, 0, :], bcm[:, 0, :], is_[:], start=True)
            mm(MG1[:, 1, :], bcm[:, 1, :], is_[:], start=True)
            nc.vector.tensor_mul(rd(y_p[:, 0:2, :]), rd(h_p[:, 0:2, :]),
                                 MG1[:])
            MG2 = pp.tile([P, 2, TILE], f32, tag="pB", name="MG2")
            mm(MG2[:, 0, :], bcm[:, 2, :], is_[:], start=True)
            mm(MG2[:, 1, :], bcm[:, 3, :], is_[:], start=True)
            nc.vector.tensor_mul(rd(y_p[:, 2:4, :]), rd(h_p[:, 2:4, :]),
                                 MG2[:])
            MG3 = pp.tile([P, TILE], f32, tag="pT", name="MG3")
            mm(MG3[:, :], bcm[:, 4, :], is_[:], start=True)
            nc.vector.tensor_mul(rd(y_p[:, 4, :]), rd(h_p[:, 4, :]), MG3[:])
            return (y_p,)

        def phaseC(t, y_p):
            yx = yxs[t % 3]
            # fusion matmul 576 -> 192; aux rhs carries bias + correction
            ZF = pp.tile([P, 2, TILE], f32, tag="pF", name="ZF")
            for s in range(5):
                mm(ZF[:, 0, :], wf1A[:, s, :], y_p[:, s, :],
                   start=(s == 0), stop=False)
            mm(ZF[:, 0, :], wf1xA[:], yx[:], start=False)
            for s in range(5):
                mm(ZF[:, 1, :], wf1B[:, s, :], y_p[:, s, :],
                   start=(s == 0), stop=False)
            mm(ZF[:, 1, :], wf1xB[:], yx[:], start=False)

            hf_p = sb.tile([P, 2, TILE], f32r, tag="hf_p", bufs=2, name="hf_p")
            nc.scalar.activation(rd(hf_p[:]), ZF[:], AF.Relu)
            hfs_p = sb.tile([P, 2, TILE], f32r, tag="hfs_p", bufs=2, name="hfs_p")
            nc.gpsimd.tensor_mul(rd(hfs_p[:]), rd(hf_p[:]), rd(hf_p[:]))
            return hf_p, hfs_p

        def phaseD(t, hf_p, hfs_p):
            yfx = yfxs[t % 2]
            # f-LN stats: SUF [2, 512] (row0 = mu, row1 = var)
            SUF = pp.tile([33, TILE], f32, tag="pS", name="SUF")
            mm(SUF[0:1, :], smf[:, 0:1], hf_p[:, 0, :], start=True,
               stop=False)
            mm(SUF[0:1, :], smf[:, 1:2], hf_p[:, 1, :], start=False)
            murf2 = sb.tile([1, TILE], f32r, tag="murf2", name="murf2")
            nc.scalar.activation(rd(murf2[:]), SUF[0:1, :], AF.Square)
            mm(SUF[32:33, :], smqf[:, 0:1], hfs_p[:, 0, :], start=True,
               stop=False)
            mm(SUF[32:33, :], smqf[:, 1:2], hfs_p[:, 1, :], start=False,
               stop=False)
            mm(SUF[32:33, :], neg5[0:1, 0:1], murf2[:], start=False)

            sef = sb.tile([1, TILE], f32, tag="sef", name="sef")
            isf = sb.tile([1, TILE], f32r, tag="isf", name="isf")
            nc.scalar.activation(sef[:], SQF[:, :], AF.Ln, bias=bcol(2, 1))
            nc.scalar.activation(rd(isf[:]), sef[:], AF.Exp, scale=-0.5,
                                 bias=bcol(3, 1))
            nc.vector.scalar_tensor_tensor(
                out=rd(yfx[0:1, :]), in0=SUF[0:1, :], scalar=1.0,
                in1=rd(isf[:]), op0=OP.mult, op1=OP.mult)

            yf_p = sb.tile([P, 2, TILE], f32r, tag="yf_p", bufs=2, name="yf_p")
            MF = pp.tile([P, 2, TILE], f32, tag="pF", name="MF")
            mm(MF[:, 0, :], bcf[:, 0, :], isf[:], start=True)
            mm(MF[:, 1, :], bcf[:, 1, :], isf[:], start=True)
            nc.vector.tensor_mul(rd(yf_p[:]), rd(hf_p[:]), MF[:])
            return (yf_p,)

        def phaseE(t, yf_p):
            yfx = yfxs[t % 2]
            # fc2: 192 -> 192 (+aux), relu
            Z2 = pp.tile([P, 2, TILE], f32, tag="pF", name="Z2")
            mm(Z2[:, 0, :], wf2A[:, 0, :], yf_p[:, 0, :], start=True,
               stop=False)
            mm(Z2[:, 0, :], wf2A[:, 1, :], yf_p[:, 1, :], start=False,
               stop=False)
            mm(Z2[:, 0, :], wf2xA[:], yfx[:], start=False)
            mm(Z2[:, 1, :], wf2B[:, 0, :], yf_p[:, 0, :], start=True,
               stop=False)
            mm(Z2[:, 1, :], wf2B[:, 1, :], yf_p[:, 1, :], start=False,
               stop=False)
            mm(Z2[:, 1, :], wf2xB[:], yfx[:], start=False)
            r2_p = sb.tile([P, 2, TILE], f32r, tag="r2_p", name="r2_p")
            nc.scalar.activation(rd(r2_p[:]), Z2[:], AF.Relu)

            # fc3: 192 -> 3 logits (row 3 unused); bf3 via const ones rhs
            ZL = pp.tile([4, TILE], f32, tag="pF", name="ZL")
            mm(ZL[:, :], wf3[:, 0, :], r2_p[:, 0, :], start=True, stop=False)
            mm(ZL[:, :], wf3[:, 1, :], r2_p[:, 1, :], start=False)
            lrow = sb.tile([4, TILE], f32, tag="lrow", bufs=2, name="lrow")
            nc.vector.tensor_copy(lrow[:], ZL[:])
            nc.sync.dma_start(out_d[:, t * TILE:(t + 1) * TILE], lrow[:])

        def whole_body(_iv=None):
            st_a = {}     # t -> (h_p, hs_p)
            st_y = {}     # t -> (y_p, yx)
            st_c = {}     # t -> (hf_p, hfs_p)
            st_e = {}     # t -> (yf_p,)
            st_b = {}
            st_g = {0: phaseG(0)}
            for t in range(n_tiles + 5):
                if t + 1 < n_tiles:
                    st_g[t + 1] = phaseG(t + 1)
                if t < n_tiles:
                    st_a[t] = phaseA(t, *st_g.pop(t))
                if t - 1 in st_a:
                    st_b[t - 1] = phaseB1(t - 1, *st_a[t - 1])
                if t - 2 in st_b:
                    st_y[t - 2] = phaseB2(t - 2, *st_a.pop(t - 2),
                                          *st_b.pop(t - 2))
                if t - 3 in st_y:
                    st_c[t - 3] = phaseC(t - 3, *st_y.pop(t - 3))
                if t - 4 in st_c:
                    st_e[t - 4] = phaseD(t - 4, *st_c.pop(t - 4))
                if t - 5 in st_e:
                    phaseE(t - 5, *st_e.pop(t - 5))

        if repeat > 1:
            with tc.For_i(0, repeat, 1):
                whole_body()
        else:
            whole_body()

    # Pin the ACT table set: keep only natural_log_exp_and_others (covers
    # Relu/Square/Ln/Exp/Copy/Identity) so the table-load pass never cycles
    # sets. Indices must stay aligned with act_info.json, so empty the other
    # sets rather than removing them.
    import concourse.bacc as _bacc_mod
    _orig_gat = _bacc_mod.get_activation_tables

    def _pinned_tables(arch):
        tabs = _orig_gat(arch)
        return {name: (s if name == "natural_log_exp_and_others" else set())
                for name, s in tabs.items()}

    _bacc_mod.get_activation_tables = _pinned_tables
    try:
        nc.compile()
    finally:
        _bacc_mod.get_activation_tables = _orig_gat
    return nc


def _get_program(n_tiles=NTILES, n_nodes=N_NODES, mmdt="bf16", repeat=1):
    key = (n_tiles, n_nodes, mmdt, repeat)
    if key not in _PROG_CACHE:
        _PROG_CACHE[key] = _build_program(n_tiles, n_nodes, mmdt, repeat)
    return _PROG_CACHE[key]


_EDGE_PERM = {"perm": None, "et": None, "bf3": None}


def _host_prep(inputs, n_tiles=NTILES, n_cores=NCORES, e_pc=E_PC,
               mmdt="bf16", n_nodes=N_NODES):
    """Fold LN gains/betas into fusion weights; build per-core input maps."""
    import ml_dtypes
    b16 = ml_dtypes.bfloat16

    f = lambda k: np.asarray(inputs[k], np.float32)
    kge = f("kge_emb")
    ei = np.asarray(inputs["edge_index"]).astype(np.int64)
    et = np.asarray(inputs["edge_type"]).astype(np.int64)
    # sort edges by src node id for gather locality; inverse perm on output
    perm = np.argsort(ei[0], kind="stable")
    _EDGE_PERM["perm"] = perm
    _EDGE_PERM["et"] = et[perm]
    _EDGE_PERM["bf3"] = np.asarray(inputs["bf3"], np.float32)
    ei = ei[:, perm]
    W1, b1, g1, be1 = f("W1"), f("b1"), f("g1"), f("be1")
    W2, b2, g2, be2 = f("W2"), f("b2"), f("g2"), f("be2")
    W3, b3, g3, be3 = f("W3"), f("b3"), f("g3"), f("be3")
    Ws, bs, gs, bes = f("Ws"), f("bs"), f("gs"), f("bes")
    Wd, bd, gd, bed = f("Wd"), f("bd"), f("gd"), f("bed")
    Wf1, bf1, gf, bef = f("Wf1"), f("bf1"), f("gf"), f("bef")
    Wf2, bf2 = f("Wf2"), f("bf2")
    Wf3, bf3 = f("Wf3"), f("bf3")

    # reference concat order: [sx, dx, b1, b2, b3]
    g_cat = np.concatenate([gs, gd, g1, g2, g3])
    be_cat = np.concatenate([bes, bed, be1, be2, be3])
    Wf1_eff = g_cat[:, None] * Wf1
    bf1_eff = bf1 + be_cat @ Wf1
    Wf2_eff = gf[:, None] * Wf2
    bf2_eff = bf2 + bef @ Wf2

    def ext(Wl, brow):
        # append the bias row to a [64, M] lower-K weight chunk
        return np.concatenate([Wl, brow[None, :]], axis=0)

    def padlo(W):
        # place into cols 0:64 of a 128-wide lhsT
        return np.concatenate([W, np.zeros_like(W)], axis=1)

    def padhi(W):
        return np.concatenate([np.zeros_like(W), W], axis=1)

    shared = {}
    shared["w1h"] = padlo(W1[0:P]); shared["w1l"] = padlo(ext(W1[P:H], b1))
    shared["w2h"] = padhi(W2[0:P]); shared["w2l"] = padhi(ext(W2[P:H], b2))
    shared["w3h"] = padlo(W3[0:P]); shared["w3l"] = padlo(ext(W3[P:H], b3))
    shared["wxh"] = padhi(Ws[0:P, P:H])
    shared["wxl"] = padhi(ext(Ws[P:H, P:H], bs[P:H]))
    shared["wsh"] = Ws[0:P, 0:P]
    shared["wsl"] = ext(Ws[P:H, 0:P], bs[0:P])
    # wd split for the packed psum groups: slice0 = dx_hi (cols 0:128),
    # slice1 = dx_lo zero-extended (cols 128:192 -> 0:64)
    wdh_s = np.zeros((P, 2, P), np.float32)
    wdh_s[:, 0, :] = Wd[0:P, 0:P]
    wdh_s[:, 1, 0:64] = Wd[0:P, P:H]
    wdl_s = np.zeros((65, 2, P), np.float32)
    wdl_s[0:64, 0, :] = Wd[P:H, 0:P]
    wdl_s[64, 0, :] = bd[0:P]
    wdl_s[0:64, 1, 0:64] = Wd[P:H, P:H]
    wdl_s[64, 1, 0:64] = bd[P:H]
    shared["wdh"] = wdh_s; shared["wdl"] = wdl_s

    # fusion weight chunks per y slice (rows of Wf1_eff):
    #   y slice0 = b1(0:64)|b2(64:128)   -> rows 384:448 | 448:512
    #   y slice1 = b3(0:64)|sx_lo(64:128)-> rows 512:576 | 128:192
    #   y slice2 = sx_hi                  -> rows 0:128
    #   y slice3 = dx_hi                  -> rows 192:320
    #   y slice4 = dx_lo(0:64)|zeros      -> rows 320:384 | -
    rows = [
        np.concatenate([Wf1_eff[384:448], Wf1_eff[448:512]]),
        np.concatenate([Wf1_eff[512:576], Wf1_eff[128:192]]),
        Wf1_eff[0:128],
        Wf1_eff[192:320],
        np.concatenate([Wf1_eff[320:384], np.zeros((64, H), np.float32)]),
    ]
    wf1A = np.stack([r[:, 0:P] for r in rows], axis=1)        # [128,5,128]
    wf1B_half = np.stack([r[:, P:H] for r in rows], axis=1)   # [128,5,64]
    wf1B = np.concatenate(
        [wf1B_half, np.zeros((P, 5, 64), np.float32)], axis=2)
    # aux: row0 = bf1_eff, rows1:6 = nc1 (order sx, dx, b1, b2, b3)
    nc1 = np.zeros((5, H), np.float32)
    cn_blocks = (H / 256.0, H / 256.0, 1.0, 1.0, 1.0)
    for b, (lo, hi) in enumerate(((0, 192), (192, 384), (384, 448),
                                  (448, 512), (512, 576))):
        nc1[b] = -Wf1_eff[lo:hi].sum(axis=0) / cn_blocks[b]
    wf1x = np.concatenate([nc1, bf1_eff[None, :]], axis=0)    # [6, 192]
    shared["wf1A"] = wf1A; shared["wf1B"] = wf1B
    shared["wf1xA"] = wf1x[:, 0:P]
    shared["wf1xB"] = np.concatenate(
        [wf1x[:, P:H], np.zeros((6, 64), np.float32)], axis=1)

    # fc2: K slices = hf slices (0: feat 0:128, 1: feat 128:192 | zeros)
    wf2A = np.zeros((P, 2, P), np.float32)
    wf2A[:, 0, :] = Wf2_eff[0:P, 0:P]
    wf2A[0:64, 1, :] = Wf2_eff[P:H, 0:P]
    wf2B = np.zeros((P, 2, P), np.float32)
    wf2B[:, 0, 0:64] = Wf2_eff[0:P, P:H]
    wf2B[0:64, 1, 0:64] = Wf2_eff[P:H, P:H]
    nc1f = -Wf2_eff.sum(axis=0) / (H / 256.0)
    wf2x = np.stack([nc1f, bf2_eff], axis=0)                  # [2, 192]
    shared["wf2A"] = wf2A; shared["wf2B"] = wf2B
    shared["wf2xA"] = wf2x[:, 0:P]
    shared["wf2xB"] = np.concatenate(
        [wf2x[:, P:H], np.zeros((2, 64), np.float32)], axis=1)

    wf3p = np.zeros((P, 2, 4), np.float32)
    wf3p[:, 0, 0:3] = Wf3[0:P]
    wf3p[0:64, 1, 0:3] = Wf3[P:H]
    shared["wf3"] = wf3p
    wf3x = np.zeros((1, 4), np.float32)
    wf3x[0, 0:3] = bf3
    shared["wf3x"] = wf3x

    # stat masks [128, 5, 5]: rows 0 sx, 1 dx, 2 b1, 3 b2, 4 b3.
    # Mask values are dyadic (exact in bf16): c = 1/256 for the n=192
    # blocks, 1/64 for the n=64 blocks; the E[h^2] masks carry c^2*n so
    # that SQ - (SU)^2 is c^2*n^2 * var, and the scale is undone through
    # the Ln/Exp bias columns (istd comes out exact) and a 1/(c*n) factor
    # folded into the nc1 correction rows.
    C3, C6 = 1.0 / 256, 1.0 / 64          # E[h] masks
    Q3, Q6 = 3.0 / 1024, 1.0 / 64         # E[h^2] masks: c^2 * n
    smu = np.zeros((P, 5, 5), np.float32)
    smq = np.zeros((P, 5, 5), np.float32)
    for (a, b_, s, k, c, q) in (
            (0, 64, 0, 2, C6, Q6), (64, 128, 0, 3, C6, Q6),
            (0, 64, 1, 4, C6, Q6), (64, 128, 1, 0, C3, Q3),
            (0, 128, 2, 0, C3, Q3), (0, 128, 3, 1, C3, Q3),
            (0, 64, 4, 1, C3, Q3)):
        smu[a:b_, s, k] = c
        smq[a:b_, s, k] = q
    shared["smu"] = smu.reshape(P, 25)
    shared["smq"] = smq.reshape(P, 25)
    smf = np.zeros((P, 2), np.float32)
    smf[:, 0] = C3
    smf[0:64, 1] = C3
    shared["smf"] = smf
    smqf = np.zeros((P, 2), np.float32)
    smqf[:, 0] = Q3
    smqf[0:64, 1] = Q3
    shared["smqf"] = smqf

    # istd broadcast masks [5, 5, 128]
    bcm = np.zeros((5, 5, P), np.float32)
    bcm[2, 0, 0:64] = 1.0    # map s0 lower <- istd b1
    bcm[3, 0, 64:128] = 1.0  # map s0 upper <- istd b2
    bcm[4, 1, 0:64] = 1.0    # map s1 lower <- istd b3
    bcm[0, 1, 64:128] = 1.0  # map s1 upper <- istd sx
    bcm[0, 2, :] = 1.0       # map s2 <- istd sx
    bcm[1, 3, :] = 1.0       # map s3 <- istd dx
    bcm[1, 4, 0:64] = 1.0    # map s4 lower <- istd dx (upper 0)
    shared["bcm"] = bcm.reshape(5, 5 * P)
    bcf = np.zeros((1, 2, P), np.float32)
    bcf[0, 0, :] = 1.0
    bcf[0, 1, 0:64] = 1.0
    shared["bcf"] = bcf.reshape(1, 2 * P)
    shared["neg5"] = -np.eye(5, dtype=np.float32)
    shared["onesr"] = np.ones((1, TILE), np.float32)
    shared["consts"] = np.eye(P, dtype=np.float32)

    # LN bias cols: 0 = k*eps rows 0:5, 1 = ln(c*n) rows 0:5,
    #               2 = kf*eps row 0, 3 = ln(cf*n) row 0
    cn = np.array([H * C3, H * C3, 64 * C6, 64 * C6, 64 * C6], np.float32)
    bias_mat = np.zeros((P, 4), np.float32)
    bias_mat[0:5, 0] = cn * cn * LN_EPS
    bias_mat[0:5, 1] = np.log(cn)
    bias_mat[0, 2] = (H * C3) ** 2 * LN_EPS
    bias_mat[0, 3] = np.log(H * C3)

    e_pad = n_tiles * TILE

    def arrange(a):
        buf = np.zeros(e_pad, a.dtype)
        buf[:e_pc] = a
        return np.ascontiguousarray(
            buf.reshape(n_tiles, CH, P).transpose(2, 0, 1).reshape(P, -1))

    f32r_keys = {"bcm", "bcf", "neg5", "onesr", "wf2A", "wf2B",
                 "wf2xA", "wf2xB", "wf3", "wf3x", "wf1A", "wf1B",
                 "wf1xA", "wf1xB", "smf", "smqf", "smu", "smq",
                 "w1h", "w1l", "w2h", "w2l", "w3h", "w3l",
                 "wxh", "wxl", "wsh", "wsl", "wdh", "wdl"}
    shared = {k: (v.astype(np.float32) if k in f32r_keys
                  else v.astype(b16)) for k, v in shared.items()}
    shared["emb"] = kge.astype(b16)
    shared["biascol"] = bias_mat

    in_maps = []
    for core in range(n_cores):
        lo = core * e_pc
        m = dict(shared)
        m["sidx"] = arrange(ei[0, lo:lo + e_pc].astype(np.int32))
        m["didx"] = arrange(ei[1, lo:lo + e_pc].astype(np.int32))
        in_maps.append(m)
    return in_maps


def _unshard(results, n_tiles=NTILES, n_cores=NCORES, e_pc=E_PC):
    # device returns logits [4, T*512]; softmax-select epilogue on host
    et = _EDGE_PERM["et"]
    bf3 = _EDGE_PERM["bf3"]
    ps = []
    for core in range(n_cores):
        lg = (np.asarray(results[core]["out"], np.float32)[0:3, :e_pc]
              + bf3[:, None])
        lg = lg - lg.max(axis=0, keepdims=True)
        ez = np.exp(lg)
        sel = np.take_along_axis(
            ez, et[core * e_pc:(core + 1) * e_pc][None, :], axis=0)[0]
        ps.append(sel / ez.sum(axis=0))
    cat = np.concatenate(ps)
    perm = _EDGE_PERM["perm"]
    if perm is not None:
        inv = np.empty_like(cat)
        inv[perm] = cat
        cat = inv
    return cat[:, None].astype(np.float32)


MMDT_MODE = "bf16"


def kernel(**inputs):
    from concourse.bass_utils import run_bass_kernel_spmd
    nc = _get_program(mmdt=MMDT_MODE)
    in_maps = _host_prep(inputs, mmdt=MMDT_MODE)
    res = run_bass_kernel_spmd(nc, in_maps, list(range(NCORES)))
    return _unshard(res.results)


# revision 56
# speedup vs baseline: 3.4284x; 1.0339x over previous
"""Trainium2 Bass kernel for DeepME edge-MLP (gnn_message_passing).

Contract: kernel(**inputs) takes FULL unsharded inputs (as produced by the
reference setup_inputs()) and returns the FULL [E, 1] float32 output.

Strategy: data-parallel over the edge dimension across 8 NeuronCores.
Embedding table and (small) MLP weights are replicated per core.

v3 device program — feature-major, bf16 compute, f32 PSUM accumulation:
  per 512-edge tile:
    one multi-column indirect DMA per side gathers 512 bf16 embedding rows
    PE-transpose to feature-major; B-halves (features 128:192) live in
      persistent [65, 512] tiles whose row 64 is a constant 1.0 so that
      layer biases ride as an extra weight row (no bias in evictions)
    branch matmuls into packed PSUM groups; 3 pure-relu evictions
      produce h [128, 5, 512] (slices: b1|b2, b3|sx_lo, sx_hi, dx_hi,
      dx_lo|0)
    LN statistics via 1/n-mask matmuls (partition sums); var = E[h^2]-mu^2
      with the mu^2 term subtracted through a -I matmul; istd broadcast to
      [128,*,512] maps by block-mask matmuls; y = h * map on the DVE
    fusion matmul consumes y slices; bias and the rank-5 -mu*istd
      correction ride on an aux [6, 512] rhs (ones row + q rows)
    fusion LN same scheme; fc2 with aux [2, 512] (ones + qf); fc3 -> 3
      logits (+bf3 via a const ones-row rhs)
    logits are DMA'd out; the 3-way softmax-select epilogue runs in
      host numpy inside kernel()
"""

import numpy as np

# ---------------------------------------------------------------------------
# problem constants (hardcoded per the harness contract)
E_TOTAL = 300000
N_NODES = 300000
H = 192
H3 = 64
NCORES = 8
P = 128
CH = 4                 # 128-edge chunks per tile
TILE = P * CH          # 512 edges per tile
E_PC = E_TOTAL // NCORES          # 37500 edges per core
NTILES = (E_PC + TILE - 1) // TILE  # 74
E_PAD = NTILES * TILE               # 37888
LN_EPS = 1e-5

_PROG_CACHE = {}


def _build_program(n_tiles, n_nodes, mmdt="bf16", repeat=1):
    """Build the SPMD Bass program. Returns the Bass object."""
    from contextlib import ExitStack

    import concourse.bass as bass
    import concourse.bacc as bacc
    import concourse.tile as tile
    import concourse.mybir as mybir

    dt = mybir.dt
    f32 = dt.float32
    i32 = dt.int32
    bf16 = dt.bfloat16
    assert mmdt == "bf16"

    f32r = dt.float32r

    def rd(ap):
        return ap.bitcast(f32) if ap.dtype == dt.float32r else ap
    AF = mybir.ActivationFunctionType
    OP = mybir.AluOpType

    ncol_idx = n_tiles * CH          # sidx/didx columns
    ncol_l = n_tiles * TILE          # logits columns ([4, T*512])

    nc = bacc.Bacc(trn_type="TRN2", target_bir_lowering=False, debug=False,
                   num_devices=NCORES)

    # ----- DRAM parameters (weights shipped pre-packed in bf16) -----------
    def din(name, shape, dtype=bf16):
        return nc.dram_tensor(name, shape, dtype, kind="ExternalInput").ap()

    emb = din("emb", [n_nodes, H])
    sidx_d = din("sidx", [P, ncol_idx], i32)
    didx_d = din("didx", [P, ncol_idx], i32)
    # branch weights: *h = K-rows 0:128, *l = K-rows 128:192 + bias row;
    # PS1 contributors are zero-padded to full 128-col outputs so no mm
    # writes at a nonzero base partition
    w1h_d = din("w1h", [P, P], f32r); w1l_d = din("w1l", [65, P], f32r)
    w2h_d = din("w2h", [P, P], f32r); w2l_d = din("w2l", [65, P], f32r)
    w3h_d = din("w3h", [P, P], f32r); w3l_d = din("w3l", [65, P], f32r)
    wxh_d = din("wxh", [P, P], f32r); wxl_d = din("wxl", [65, P], f32r)
    wsh_d = din("wsh", [P, P], f32r); wsl_d = din("wsl", [65, P], f32r)
    wdh_d = din("wdh", [P, 2, P], f32r); wdl_d = din("wdl", [65, 2, P], f32r)
    # fusion: per y-slice lhsT chunks for out slice0 (feat 0:128) and
    # slice1 (feat 128:192 zero-extended), plus aux [6, 128] x2
    wf1A_d = din("wf1A", [P, 5, P], f32r)
    wf1B_d = din("wf1B", [P, 5, P], f32r)
    wf1xA_d = din("wf1xA", [6, P], f32r)
    wf1xB_d = din("wf1xB", [6, P], f32r)
    wf2A_d = din("wf2A", [P, 2, P], f32r)
    wf2B_d = din("wf2B", [P, 2, P], f32r)
    wf2xA_d = din("wf2xA", [2, P], f32r)
    wf2xB_d = din("wf2xB", [2, P], f32r)
    wf3_d = din("wf3", [P, 2, 4], f32r)
    wf3x_d = din("wf3x", [1, 4], f32r)
    # masks: stat masks [128,5,5] + [1x...] packed, bcast masks, -I5
    smu_d = din("smu", [P, 5 * 5], f32r)  # per-h-slice E[h] stat masks
    smq_d = din("smq", [P, 5 * 5], f32r)  # per-h-slice E[h^2] stat masks
    smf_d = din("smf", [P, 2], f32r)      # f-LN E[h] stat masks
    smqf_d = din("smqf", [P, 2], f32r)    # f-LN E[h^2] stat masks
    bcm_d = din("bcm", [5, 5 * P], f32r)  # istd bcast masks per slice
    bcf_d = din("bcf", [1, 2 * P], f32r)  # f-LN bcast masks
    neg5_d = din("neg5", [5, 5], f32r)    # -I5
    onesr_d = din("onesr", [1, TILE], f32r)  # const ones row (bf3 rhs)
    bias_d = din("biascol", [P, 4], f32)  # LN bias columns
    cst_d = din("consts", [P, P])         # bf16 identity
    out_d = nc.dram_tensor("out", [4, ncol_l], f32, kind="ExternalOutput").ap()

    def mm(out, lhsT, rhs, start, stop=True):
        nc.tensor.matmul(out=out, lhsT=lhsT, rhs=rhs, start=start, stop=stop)

    with tile.TileContext(nc) as tc, ExitStack() as ctx:
        cpool = ctx.enter_context(tc.tile_pool(name="const", bufs=1))
        sb = ctx.enter_context(tc.tile_pool(name="work", bufs=1))
        sb2 = ctx.enter_context(tc.tile_pool(name="work2", bufs=2))
        pp = ctx.enter_context(tc.tile_pool(name="psum", bufs=1, space="PSUM"))

        # ----- resident tiles (loaded once) -------------------------------
        def cload(shape, src_ap, name, dtype=bf16):
            t_ = cpool.tile(shape, dtype, name=name)
            nc.sync.dma_start(t_[:], src_ap)
            return t_

        sidx = cload([P, ncol_idx], sidx_d[:], "sidx", i32)
        didx = cload([P, ncol_idx], didx_d[:], "didx", i32)
        w1h = cload([P, P], w1h_d[:], "w1h", f32r)
        w1l = cload([65, P], w1l_d[:], "w1l", f32r)
        w2h = cload([P, P], w2h_d[:], "w2h", f32r)
        w2l = cload([65, P], w2l_d[:], "w2l", f32r)
        w3h = cload([P, P], w3h_d[:], "w3h", f32r)
        w3l = cload([65, P], w3l_d[:], "w3l", f32r)
        wxh = cload([P, P], wxh_d[:], "wxh", f32r)
        wxl = cload([65, P], wxl_d[:], "wxl", f32r)
        wsh = cload([P, P], wsh_d[:], "wsh", f32r)
        wsl = cload([65, P], wsl_d[:], "wsl", f32r)
        wdh = cload([P, 2, P], wdh_d[:], "wdh", f32r)
        wdl = cload([65, 2, P], wdl_d[:], "wdl", f32r)
        wf1A = cload([P, 5, P], wf1A_d[:], "wf1A", f32r)
        wf1B = cload([P, 5, P], wf1B_d[:], "wf1B", f32r)
        wf1xA = cload([6, P], wf1xA_d[:], "wf1xA", f32r)
        wf1xB = cload([6, P], wf1xB_d[:], "wf1xB", f32r)
        wf2A = cload([P, 2, P], wf2A_d[:], "wf2A", f32r)
        wf2B = cload([P, 2, P], wf2B_d[:], "wf2B", f32r)
        wf2xA = cload([2, P], wf2xA_d[:], "wf2xA", f32r)
        wf2xB = cload([2, P], wf2xB_d[:], "wf2xB", f32r)
        wf3 = cload([P, 2, 4], wf3_d[:], "wf3", f32r)
        wf3x = cload([1, 4], wf3x_d[:], "wf3x", f32r)
        smu_t = cload([P, 25], smu_d[:], "smu", f32r)
        smu = smu_t[:].rearrange("p (s k) -> p s k", k=5)       # [128,5,5]
        smq_t = cload([P, 25], smq_d[:], "smq", f32r)
        smq = smq_t[:].rearrange("p (s k) -> p s k", k=5)       # [128,5,5]
        smf = cload([P, 2], smf_d[:], "smf", f32r)
        smqf = cload([P, 2], smqf_d[:], "smqf", f32r)
        bcm_t = cload([5, 5 * P], bcm_d[:], "bcm", f32r)
        bcm = bcm_t[:].rearrange("p (s m) -> p s m", m=P)       # [5,5,128]
        bcf_t = cload([1, 2 * P], bcf_d[:], "bcf", f32r)
        bcf = bcf_t[:].rearrange("p (s m) -> p s m", m=P)       # [1,2,128]
        neg5 = cload([5, 5], neg5_d[:], "neg5", f32r)
        onesr = cload([1, TILE], onesr_d[:], "onesr", f32r)
        bias = cload([P, 4], bias_d[:], "bias", f32)
        cst = cload([P, P], cst_d[:], "ident")
        ident = cst[:]

        def tp(out, in_):
            k = in_.partition_size()
            nc.tensor.transpose(out=out, in_=in_, identity=ident[0:k, 0:k])

        # bias cols: 0 = k*eps rows 0:5, 1 = ln(c*n) rows 0:5,
        #            2 = kf*eps row 0,   3 = ln(cf*n) row 0
        def bcol(j, np_=P):
            return bias[0:np_, j:j + 1]

        # persistent [65, 512] operand tiles: row 64 is a constant 1.0 so
        # bias rows in the *l weights apply; writers only touch rows 0:64
        def ones_row_tile(name):
            t_ = cpool.tile([65, TILE], f32r, name=name)
            nc.vector.memset(rd(t_[64:65, :]), 1.0)
            return t_

        srcTB = ones_row_tile("srcTB")
        dstTB = ones_row_tile("dstTB")
        difB = ones_row_tile("difB")
        prdB = ones_row_tile("prdB")
        sqB = ones_row_tile("sqB")
        # aux fusion rhs: rows 0:5 = q (written per tile), row 5 = 1
        yx0 = cpool.tile([6, TILE], f32r, name="yx0")
        yx1 = cpool.tile([6, TILE], f32r, name="yx1")
        yx2 = cpool.tile([6, TILE], f32r, name="yx2")
        nc.vector.memset(rd(yx0[:, :]), 1.0)
        nc.vector.memset(rd(yx1[:, :]), 1.0)
        nc.vector.memset(rd(yx2[:, :]), 1.0)
        yxs = [yx0, yx1, yx2]
        # aux fc2 rhs: row0 = qf, row1 = 1
        yfx0 = cpool.tile([2, TILE], f32r, name="yfx0")
        yfx1 = cpool.tile([2, TILE], f32r, name="yfx1")
        nc.vector.memset(rd(yfx0[:, :]), 1.0)
        nc.vector.memset(rd(yfx1[:, :]), 1.0)
        yfxs = [yfx0, yfx1]

        # ----- five-phase pipelined tile loop -----------------------------
        # A(t): gather, transpose, elementwise, branch matmuls, evictions,
        #       squares
        # B(t): stat matmuls, istd, bcast maps, y = h*map, q rows
        # C(t): fusion matmuls, eviction, squares
        # D(t): f-LN stats, istd_f, map, yf, qf
        # E(t): fc2, eviction, fc3 logits, copy out, DMA
        # emission: A(t) C(t-1) B(t) D(t-1) E(t-1)

        def phaseG(t):
            srcG = sb2.tile([P, CH, H], bf16, tag="srcG", name="srcG")
            dstG = sb2.tile([P, CH, H], bf16, tag="dstG", name="dstG")
            for c in range(CH):
                cc = slice(t * CH + c, t * CH + c + 1)
                nc.gpsimd.indirect_dma_start(
                    out=srcG[:, c, :], out_offset=None, in_=emb[:, :],
                    in_offset=bass.IndirectOffsetOnAxis(ap=sidx[:, cc],
                                                        axis=0))
                nc.gpsimd.indirect_dma_start(
                    out=dstG[:, c, :], out_offset=None, in_=emb[:, :],
                    in_offset=bass.IndirectOffsetOnAxis(ap=didx[:, cc],
                                                        axis=0))
            return srcG, dstG

        def phaseA(t, srcG, dstG):
            # transposes through a 2KB psum tag, src then dst
            sTs = pp.tile([P, 2, TILE], bf16, tag="pT", name="sTs")
            for c in range(CH):
                cs = slice(c * P, (c + 1) * P)
                tp(sTs[:, 0, cs], srcG[:, c, 0:P])
                tp(sTs[0:64, 1, cs], srcG[:, c, P:H])
            srcTA = sb.tile([P, TILE], f32r, tag="srcTA", name="srcTA")
            dstTA = sb.tile([P, TILE], f32r, tag="dstTA", name="dstTA")
            nc.any.tensor_copy(rd(srcTA[:]), sTs[:, 0, :])
            nc.any.tensor_copy(rd(srcTB[0:64, :]), sTs[0:64, 1, :])
            sTd = pp.tile([P, 2, TILE], bf16, tag="pT", name="sTd")
            for c in range(CH):
                cs = slice(c * P, (c + 1) * P)
                tp(sTd[:, 0, cs], dstG[:, c, 0:P])
                tp(sTd[0:64, 1, cs], dstG[:, c, P:H])
            nc.any.tensor_copy(rd(dstTA[:]), sTd[:, 0, :])
            nc.any.tensor_copy(rd(dstTB[0:64, :]), sTd[0:64, 1, :])

            # elementwise: diff, prod, diff^2 (A on DVE, B on gpsimd)
            difA = sb.tile([P, TILE], f32r, tag="difA", name="difA")
            prdA = sb.tile([P, TILE], f32r, tag="prdA", name="prdA")
            sqA = sb.tile([P, TILE], f32r, tag="sqA", name="sqA")
            nc.vector.tensor_sub(rd(difA[:]), rd(srcTA[:]), rd(dstTA[:]))
            nc.vector.tensor_mul(rd(prdA[:]), rd(srcTA[:]), rd(dstTA[:]))
            nc.scalar.activation(rd(sqA[:]), rd(difA[:]), AF.Square)
            nc.gpsimd.tensor_sub(rd(difB[0:64, :]), rd(srcTB[0:64, :]),
                                 rd(dstTB[0:64, :]))
            nc.gpsimd.tensor_mul(rd(prdB[0:64, :]), rd(srcTB[0:64, :]),
                                 rd(dstTB[0:64, :]))
            nc.gpsimd.tensor_mul(rd(sqB[0:64, :]), rd(difB[0:64, :]),
                                 rd(difB[0:64, :]))

            # branch matmuls into packed PSUM groups (biases ride on the
            # ones row of the B operands)
            # PS1 [128,2,512]: s0 = b1(0:64) + b2(64:128); s1 = b3 + sx_lo
            # PS2 [128,2,512]: s0 = sx_hi;  s1 = dx_hi
            # PS3 [128,512]:   dx_lo zero-extended
            PS1 = pp.tile([P, 2, TILE], f32, tag="pA", name="PS1")
            mm(PS1[:, 0, :], w1h[:], difA[:], start=True, stop=False)
            mm(PS1[:, 0, :], w1l[:], difB[:], start=False, stop=False)
            mm(PS1[:, 0, :], w2h[:], sqA[:], start=False, stop=False)
            mm(PS1[:, 0, :], w2l[:], sqB[:], start=False)
            mm(PS1[:, 1, :], w3h[:], prdA[:], start=True, stop=False)
            mm(PS1[:, 1, :], w3l[:], prdB[:], start=False, stop=False)
            mm(PS1[:, 1, :], wxh[:], srcTA[:], start=False, stop=False)
            mm(PS1[:, 1, :], wxl[:], srcTB[:], start=False)
            PS2 = pp.tile([P, 2, TILE], f32, tag="pB", name="PS2")
            mm(PS2[:, 0, :], wsh[:], srcTA[:], start=True, stop=False)
            mm(PS2[:, 0, :], wsl[:], srcTB[:], start=False)
            mm(PS2[:, 1, :], wdh[:, 0, :], dstTA[:], start=True, stop=False)
            mm(PS2[:, 1, :], wdl[:, 0, :], dstTB[:], start=False)
            PS3 = pp.tile([P, TILE], f32, tag="pF", name="PS3")
            mm(PS3[:, :], wdh[:, 1, :], dstTA[:], start=True, stop=False)
            mm(PS3[:, :], wdl[:, 1, :], dstTB[:], start=False)

            # pure-relu evictions into h [128, 5, 512]
            # h slices: 0 = b1|b2, 1 = b3|sx_lo, 2 = sx_hi, 3 = dx_hi,
            #           4 = dx_lo|zeros
            h_p = sb.tile([P, 5, TILE], f32r, tag="h_p", bufs=3, name="h_p")
            hs_p = sb.tile([P, 5, TILE], f32r, tag="hs_p", bufs=3, name="hs_p")
            nc.scalar.activation(rd(h_p[:, 0:2, :]), PS1[:], AF.Relu)
            nc.vector.tensor_mul(rd(hs_p[:, 0:2, :]), rd(h_p[:, 0:2, :]),
                                 rd(h_p[:, 0:2, :]))
            nc.scalar.activation(rd(h_p[:, 2:4, :]), PS2[:], AF.Relu)
            nc.scalar.activation(rd(h_p[:, 4, :]), PS3[:], AF.Relu)
            nc.gpsimd.tensor_mul(rd(hs_p[:, 2:4, :]), rd(h_p[:, 2:4, :]),
                                 rd(h_p[:, 2:4, :]))
            nc.gpsimd.tensor_mul(rd(hs_p[:, 4, :]), rd(h_p[:, 4, :]),
                                 rd(h_p[:, 4, :]))
            return h_p, hs_p

        def phaseB1(t, h_p, hs_p):
            # stat matmuls; masks carry 1/n -> SU = mu, SQ = var (after the
            # -I5 * mu^2 accumulation); SUQ packed [10, 512]
            SUQ = pp.tile([37, TILE], f32, tag="pS", name="SUQ")
            for s in range(5):
                mm(SUQ[0:5, :], smu[:, s, :], h_p[:, s, :],
                   start=(s == 0), stop=(s == 4))
            mur2 = sb.tile([5, TILE], f32r, tag="mur2", name="mur2")
            nc.scalar.activation(rd(mur2[:]), SUQ[0:5, :], AF.Square)
            for s in range(5):
                mm(SUQ[32:37, :], smq[:, s, :], hs_p[:, s, :],
                   start=(s == 0), stop=False)
            mm(SUQ[32:37, :], neg5[:], mur2[:], start=False)

            # istd rows: is = exp(-0.5 ln(var + eps)); q = mu * is
            se = sb.tile([5, TILE], f32, tag="se", name="se")
            is_ = sb.tile([5, TILE], f32r, tag="is_", bufs=2, name="is_")
            nc.scalar.activation(se[:], SQ[:, :], AF.Ln, bias=bcol(0, 5))
            nc.scalar.activation(rd(is_[:]), se[:], AF.Exp, scale=-0.5,
                                 bias=bcol(1, 5))
            yx = yxs[t % 3]
            nc.vector.scalar_tensor_tensor(
                out=rd(yx[0:5, :]), in0=SUQ[0:5, :], scalar=1.0,
                in1=rd(is_[:]),
                op0=OP.mult, op1=OP.mult)
            return (is_,)

        def phaseB2(t, h_p, hs_p, is_):
            # istd maps + y = h * map; map groups through psum tags
            y_p = sb.tile([P, 5, TILE], f32r, tag="y_p", bufs=2, name="y_p")
            MG1 = pp.tile([P, 2, TILE], f32, tag="pA", name="MG1")
            mm(MG1[:, 0, :], bcm[:, 0, :], is_[:], start=True)
            mm(MG1[:, 1, :], bcm[:, 1, :], is_[:], start=True)
            nc.vector.tensor_mul(rd(y_p[:, 0:2, :]), rd(h_p[:, 0:2, :]),
                                 MG1[:])
            MG2 = pp.tile([P, 2, TILE], f32, tag="pB", name="MG2")
            mm(MG2[:, 0, :], bcm[:, 2, :], is_[:], start=True)
            mm(MG2[:, 1, :], bcm[:, 3, :], is_[:], start=True)
            nc.vector.tensor_mul(rd(y_p[:, 2:4, :]), rd(h_p[:, 2:4, :]),
                                 MG2[:])
            MG3 = pp.tile([P, TILE], f32, tag="pT", name="MG3")
            mm(MG3[:, :], bcm[:, 4, :], is_[:], start=True)
            nc.vector.tensor_mul(rd(y_p[:, 4, :]), rd(h_p[:, 4, :]), MG3[:])
            return (y_p,)

        def phaseC(t, y_p):
            yx = yxs[t % 3]
            # fusion matmul 576 -> 192; aux rhs carries bias + correction
            ZF = pp.tile([P, 2, TILE], f32, tag="pF", name="ZF")
            for s in range(5):
                mm(ZF[:, 0, :], wf1A[:, s, :], y_p[:, s, :],
                   start=(s == 0), stop=False)
            mm(ZF[:, 0, :], wf1xA[:], yx[:], start=False)
            for s in range(5):
                mm(ZF[:, 1, :], wf1B[:, s, :], y_p[:, s, :],
                   start=(s == 0), stop=False)
            mm(ZF[:, 1, :], wf1xB[:], yx[:], start=False)

            hf_p = sb.tile([P, 2, TILE], f32r, tag="hf_p", bufs=2, name="hf_p")
            nc.scalar.activation(rd(hf_p[:]), ZF[:], AF.Relu)
            hfs_p = sb.tile([P, 2, TILE], f32r, tag="hfs_p", bufs=2, name="hfs_p")
            nc.gpsimd.tensor_mul(rd(hfs_p[:]), rd(hf_p[:]), rd(hf_p[:]))
            return hf_p, hfs_p

        def phaseD(t, hf_p, hfs_p):
            yfx = yfxs[t % 2]
            # f-LN stats: SUF [2, 512] (row0 = mu, row1 = var)
            SUF = pp.tile([33, TILE], f32, tag="pS", name="SUF")
            mm(SUF[0:1, :], smf[:, 0:1], hf_p[:, 0, :], start=True,
               stop=False)
            mm(SUF[0:1, :], smf[:, 1:2], hf_p[:, 1, :], start=False)
            murf2 = sb.tile([1, TILE], f32r, tag="murf2", name="murf2")
            nc.scalar.activation(rd(murf2[:]), SUF[0:1, :], AF.Square)
            mm(SUF[32:33, :], smqf[:, 0:1], hfs_p[:, 0, :], start=True,
               stop=False)
            mm(SUF[32:33, :], smqf[:, 1:2], hfs_p[:, 1, :], start=False,
               stop=False)
            mm(SUF[32:33, :], neg5[0:1, 0:1], murf2[:], start=False)

            sef = sb.tile([1, TILE], f32, tag="sef", name="sef")
            isf = sb.tile([1, TILE], f32r, tag="isf", name="isf")
            nc.scalar.activation(sef[:], SQF[:, :], AF.Ln, bias=bcol(2, 1))
            nc.scalar.activation(rd(isf[:]), sef[:], AF.Exp, scale=-0.5,
                                 bias=bcol(3, 1))
            nc.vector.scalar_tensor_tensor(
                out=rd(yfx[0:1, :]), in0=SUF[0:1, :], scalar=1.0,
                in1=rd(isf[:]), op0=OP.mult, op1=OP.mult)

            yf_p = sb.tile([P, 2, TILE], f32r, tag="yf_p", bufs=2, name="yf_p")
            MF = pp.tile([P, 2, TILE], f32, tag="pF", name="MF")
            mm(MF[:, 0, :], bcf[:, 0, :], isf[:], start=True)
            mm(MF[:, 1, :], bcf[:, 1, :], isf[:], start=True)
            nc.vector.tensor_mul(rd(yf_p[:]), rd(hf_p[:]), MF[:])
            return (yf_p,)

        def phaseE(t, yf_p):
            yfx = yfxs[t % 2]
            # fc2: 192 -> 192 (+aux), relu
            Z2 = pp.tile([P, 2, TILE], f32, tag="pF", name="Z2")
            mm(Z2[:, 0, :], wf2A[:, 0, :], yf_p[:, 0, :], start=True,
               stop=False)
            mm(Z2[:, 0, :], wf2A[:, 1, :], yf_p[:, 1, :], start=False,
               stop=False)
            mm(Z2[:, 0, :], wf2xA[:], yfx[:], start=False)
            mm(Z2[:, 1, :], wf2B[:, 0, :], yf_p[:, 0, :], start=True,
               stop=False)
            mm(Z2[:, 1, :], wf2B[:, 1, :], yf_p[:, 1, :], start=False,
               stop=False)
            mm(Z2[:, 1, :], wf2xB[:], yfx[:], start=False)
            r2_p = sb.tile([P, 2, TILE], f32r, tag="r2_p", name="r2_p")
            nc.scalar.activation(rd(r2_p[:]), Z2[:], AF.Relu)

            # fc3: 192 -> 3 logits (row 3 unused); bf3 via const ones rhs
            ZL = pp.tile([4, TILE], f32, tag="pF", name="ZL")
            mm(ZL[:, :], wf3[:, 0, :], r2_p[:, 0, :], start=True, stop=False)
            mm(ZL[:, :], wf3[:, 1, :], r2_p[:, 1, :], start=False,
               stop=False)
            mm(ZL[:, :], wf3x[:], onesr[:], start=False)
            lrow = sb.tile([4, TILE], f32, tag="lrow", bufs=2, name="lrow")
            nc.vector.tensor_copy(lrow[:], ZL[:])
            nc.sync.dma_start(out_d[:, t * TILE:(t + 1) * TILE], lrow[:])

        def whole_body(_iv=None):
            st_a = {}     # t -> (h_p, hs_p)
            st_y = {}     # t -> (y_p, yx)
            st_c = {}     # t -> (hf_p, hfs_p)
            st_e = {}     # t -> (yf_p,)
            st_b = {}
            st_g = {0: phaseG(0)}
            for t in range(n_tiles + 5):
                if t + 1 < n_tiles:
                    st_g[t + 1] = phaseG(t + 1)
                if t < n_tiles:
                    st_a[t] = phaseA(t, *st_g.pop(t))
                if t - 1 in st_a:
                    st_b[t - 1] = phaseB1(t - 1, *st_a[t - 1])
                if t - 2 in st_b:
                    st_y[t - 2] = phaseB2(t - 2, *st_a.pop(t - 2),
                                          *st_b.pop(t - 2))
                if t - 3 in st_y:
                    st_c[t - 3] = phaseC(t - 3, *st_y.pop(t - 3))
                if t - 4 in st_c:
                    st_e[t - 4] = phaseD(t - 4, *st_c.pop(t - 4))
                if t - 5 in st_e:
                    phaseE(t - 5, *st_e.pop(t - 5))

        if repeat > 1:
            with tc.For_i(0, repeat, 1):
                whole_body()
        else:
            whole_body()

    # Pin the ACT table set: keep only natural_log_exp_and_others (covers
    # Relu/Square/Ln/Exp/Copy/Identity) so the table-load pass never cycles
    # sets. Indices must stay aligned with act_info.json, so empty the other
    # sets rather than removing them.
    import concourse.bacc as _bacc_mod
    _orig_gat = _bacc_mod.get_activation_tables

    def _pinned_tables(arch):
        tabs = _orig_gat(arch)
        return {name: (s if name == "natural_log_exp_and_others" else set())
                for name, s in tabs.items()}

    _bacc_mod.get_activation_tables = _pinned_tables
    try:
        nc.compile()
    finally:
        _bacc_mod.get_activation_tables = _orig_gat
    return nc


def _get_program(n_tiles=NTILES, n_nodes=N_NODES, mmdt="bf16", repeat=1):
    key = (n_tiles, n_nodes, mmdt, repeat)
    if key not in _PROG_CACHE:
        _PROG_CACHE[key] = _build_program(n_tiles, n_nodes, mmdt, repeat)
    return _PROG_CACHE[key]


_EDGE_PERM = {"perm": None, "et": None}


def _host_prep(inputs, n_tiles=NTILES, n_cores=NCORES, e_pc=E_PC,
               mmdt="bf16", n_nodes=N_NODES):
    """Fold LN gains/betas into fusion weights; build per-core input maps."""
    import ml_dtypes
    b16 = ml_dtypes.bfloat16

    f = lambda k: np.asarray(inputs[k], np.float32)
    kge = f("kge_emb")
    ei = np.asarray(inputs["edge_index"]).astype(np.int64)
    et = np.asarray(inputs["edge_type"]).astype(np.int64)
    # sort edges by src node id for gather locality; inverse perm on output
    perm = np.argsort(ei[0], kind="stable")
    _EDGE_PERM["perm"] = perm
    _EDGE_PERM["et"] = et[perm]
    ei = ei[:, perm]
    W1, b1, g1, be1 = f("W1"), f("b1"), f("g1"), f("be1")
    W2, b2, g2, be2 = f("W2"), f("b2"), f("g2"), f("be2")
    W3, b3, g3, be3 = f("W3"), f("b3"), f("g3"), f("be3")
    Ws, bs, gs, bes = f("Ws"), f("bs"), f("gs"), f("bes")
    Wd, bd, gd, bed = f("Wd"), f("bd"), f("gd"), f("bed")
    Wf1, bf1, gf, bef = f("Wf1"), f("bf1"), f("gf"), f("bef")
    Wf2, bf2 = f("Wf2"), f("bf2")
    Wf3, bf3 = f("Wf3"), f("bf3")

    # reference concat order: [sx, dx, b1, b2, b3]
    g_cat = np.concatenate([gs, gd, g1, g2, g3])
    be_cat = np.concatenate([bes, bed, be1, be2, be3])
    Wf1_eff = g_cat[:, None] * Wf1
    bf1_eff = bf1 + be_cat @ Wf1
    Wf2_eff = gf[:, None] * Wf2
    bf2_eff = bf2 + bef @ Wf2

    def ext(Wl, brow):
        # append the bias row to a [64, M] lower-K weight chunk
        return np.concatenate([Wl, brow[None, :]], axis=0)

    def padlo(W):
        # place into cols 0:64 of a 128-wide lhsT
        return np.concatenate([W, np.zeros_like(W)], axis=1)

    def padhi(W):
        return np.concatenate([np.zeros_like(W), W], axis=1)

    shared = {}
    shared["w1h"] = padlo(W1[0:P]); shared["w1l"] = padlo(ext(W1[P:H], b1))
    shared["w2h"] = padhi(W2[0:P]); shared["w2l"] = padhi(ext(W2[P:H], b2))
    shared["w3h"] = padlo(W3[0:P]); shared["w3l"] = padlo(ext(W3[P:H], b3))
    shared["wxh"] = padhi(Ws[0:P, P:H])
    shared["wxl"] = padhi(ext(Ws[P:H, P:H], bs[P:H]))
    shared["wsh"] = Ws[0:P, 0:P]
    shared["wsl"] = ext(Ws[P:H, 0:P], bs[0:P])
    # wd split for the packed psum groups: slice0 = dx_hi (cols 0:128),
    # slice1 = dx_lo zero-extended (cols 128:192 -> 0:64)
    wdh_s = np.zeros((P, 2, P), np.float32)
    wdh_s[:, 0, :] = Wd[0:P, 0:P]
    wdh_s[:, 1, 0:64] = Wd[0:P, P:H]
    wdl_s = np.zeros((65, 2, P), np.float32)
    wdl_s[0:64, 0, :] = Wd[P:H, 0:P]
    wdl_s[64, 0, :] = bd[0:P]
    wdl_s[0:64, 1, 0:64] = Wd[P:H, P:H]
    wdl_s[64, 1, 0:64] = bd[P:H]
    shared["wdh"] = wdh_s; shared["wdl"] = wdl_s

    # fusion weight chunks per y slice (rows of Wf1_eff):
    #   y slice0 = b1(0:64)|b2(64:128)   -> rows 384:448 | 448:512
    #   y slice1 = b3(0:64)|sx_lo(64:128)-> rows 512:576 | 128:192
    #   y slice2 = sx_hi                  -> rows 0:128
    #   y slice3 = dx_hi                  -> rows 192:320
    #   y slice4 = dx_lo(0:64)|zeros      -> rows 320:384 | -
    rows = [
        np.concatenate([Wf1_eff[384:448], Wf1_eff[448:512]]),
        np.concatenate([Wf1_eff[512:576], Wf1_eff[128:192]]),
        Wf1_eff[0:128],
        Wf1_eff[192:320],
        np.concatenate([Wf1_eff[320:384], np.zeros((64, H), np.float32)]),
    ]
    wf1A = np.stack([r[:, 0:P] for r in rows], axis=1)        # [128,5,128]
    wf1B_half = np.stack([r[:, P:H] for r in rows], axis=1)   # [128,5,64]
    wf1B = np.concatenate(
        [wf1B_half, np.zeros((P, 5, 64), np.float32)], axis=2)
    # aux: row0 = bf1_eff, rows1:6 = nc1 (order sx, dx, b1, b2, b3)
    nc1 = np.zeros((5, H), np.float32)
    cn_blocks = (H / 256.0, H / 256.0, 1.0, 1.0, 1.0)
    for b, (lo, hi) in enumerate(((0, 192), (192, 384), (384, 448),
                                  (448, 512), (512, 576))):
        nc1[b] = -Wf1_eff[lo:hi].sum(axis=0) / cn_blocks[b]
    wf1x = np.concatenate([nc1, bf1_eff[None, :]], axis=0)    # [6, 192]
    shared["wf1A"] = wf1A; shared["wf1B"] = wf1B
    shared["wf1xA"] = wf1x[:, 0:P]
    shared["wf1xB"] = np.concatenate(
        [wf1x[:, P:H], np.zeros((6, 64), np.float32)], axis=1)

    # fc2: K slices = hf slices (0: feat 0:128, 1: feat 128:192 | zeros)
    wf2A = np.zeros((P, 2, P), np.float32)
    wf2A[:, 0, :] = Wf2_eff[0:P, 0:P]
    wf2A[0:64, 1, :] = Wf2_eff[P:H, 0:P]
    wf2B = np.zeros((P, 2, P), np.float32)
    wf2B[:, 0, 0:64] = Wf2_eff[0:P, P:H]
    wf2B[0:64, 1, 0:64] = Wf2_eff[P:H, P:H]
    nc1f = -Wf2_eff.sum(axis=0) / (H / 256.0)
    wf2x = np.stack([nc1f, bf2_eff], axis=0)                  # [2, 192]
    shared["wf2A"] = wf2A; shared["wf2B"] = wf2B
    shared["wf2xA"] = wf2x[:, 0:P]
    shared["wf2xB"] = np.concatenate(
        [wf2x[:, P:H], np.zeros((2, 64), np.float32)], axis=1)

    wf3p = np.zeros((P, 2, 4), np.float32)
    wf3p[:, 0, 0:3] = Wf3[0:P]
    wf3p[0:64, 1, 0:3] = Wf3[P:H]
    shared["wf3"] = wf3p
    wf3x = np.zeros((1, 4), np.float32)
    wf3x[0, 0:3] = bf3
    shared["wf3x"] = wf3x

    # stat masks [128, 5, 5]: rows 0 sx, 1 dx, 2 b1, 3 b2, 4 b3.
    # Mask values are dyadic (exact in bf16): c = 1/256 for the n=192
    # blocks, 1/64 for the n=64 blocks; the E[h^2] masks carry c^2*n so
    # that SQ - (SU)^2 is c^2*n^2 * var, and the scale is undone through
    # the Ln/Exp bias columns (istd comes out exact) and a 1/(c*n) factor
    # folded into the nc1 correction rows.
    C3, C6 = 1.0 / 256, 1.0 / 64          # E[h] masks
    Q3, Q6 = 3.0 / 1024, 1.0 / 64         # E[h^2] masks: c^2 * n
    smu = np.zeros((P, 5, 5), np.float32)
    smq = np.zeros((P, 5, 5), np.float32)
    for (a, b_, s, k, c, q) in (
            (0, 64, 0, 2, C6, Q6), (64, 128, 0, 3, C6, Q6),
            (0, 64, 1, 4, C6, Q6), (64, 128, 1, 0, C3, Q3),
            (0, 128, 2, 0, C3, Q3), (0, 128, 3, 1, C3, Q3),
            (0, 64, 4, 1, C3, Q3)):
        smu[a:b_, s, k] = c
        smq[a:b_, s, k] = q
    shared["smu"] = smu.reshape(P, 25)
    shared["smq"] = smq.reshape(P, 25)
    smf = np.zeros((P, 2), np.float32)
    smf[:, 0] = C3
    smf[0:64, 1] = C3
    shared["smf"] = smf
    smqf = np.zeros((P, 2), np.float32)
    smqf[:, 0] = Q3
    smqf[0:64, 1] = Q3
    shared["smqf"] = smqf

    # istd broadcast masks [5, 5, 128]
    bcm = np.zeros((5, 5, P), np.float32)
    bcm[2, 0, 0:64] = 1.0    # map s0 lower <- istd b1
    bcm[3, 0, 64:128] = 1.0  # map s0 upper <- istd b2
    bcm[4, 1, 0:64] = 1.0    # map s1 lower <- istd b3
    bcm[0, 1, 64:128] = 1.0  # map s1 upper <- istd sx
    bcm[0, 2, :] = 1.0       # map s2 <- istd sx
    bcm[1, 3, :] = 1.0       # map s3 <- istd dx
    bcm[1, 4, 0:64] = 1.0    # map s4 lower <- istd dx (upper 0)
    shared["bcm"] = bcm.reshape(5, 5 * P)
    bcf = np.zeros((1, 2, P), np.float32)
    bcf[0, 0, :] = 1.0
    bcf[0, 1, 0:64] = 1.0
    shared["bcf"] = bcf.reshape(1, 2 * P)
    shared["neg5"] = -np.eye(5, dtype=np.float32)
    shared["onesr"] = np.ones((1, TILE), np.float32)
    shared["consts"] = np.eye(P, dtype=np.float32)

    # LN bias cols: 0 = k*eps rows 0:5, 1 = ln(c*n) rows 0:5,
    #               2 = kf*eps row 0, 3 = ln(cf*n) row 0
    cn = np.array([H * C3, H * C3, 64 * C6, 64 * C6, 64 * C6], np.float32)
    bias_mat = np.zeros((P, 4), np.float32)
    bias_mat[0:5, 0] = cn * cn * LN_EPS
    bias_mat[0:5, 1] = np.log(cn)
    bias_mat[0, 2] = (H * C3) ** 2 * LN_EPS
    bias_mat[0, 3] = np.log(H * C3)

    e_pad = n_tiles * TILE

    def arrange(a):
        buf = np.zeros(e_pad, a.dtype)
        buf[:e_pc] = a
        return np.ascontiguousarray(
            buf.reshape(n_tiles, CH, P).transpose(2, 0, 1).reshape(P, -1))

    f32r_keys = {"bcm", "bcf", "neg5", "onesr", "wf2A", "wf2B",
                 "wf2xA", "wf2xB", "wf3", "wf3x", "wf1A", "wf1B",
                 "wf1xA", "wf1xB", "smf", "smqf", "smu", "smq",
                 "w1h", "w1l", "w2h", "w2l", "w3h", "w3l",
                 "wxh", "wxl", "wsh", "wsl", "wdh", "wdl"}
    shared = {k: (v.astype(np.float32) if k in f32r_keys
                  else v.astype(b16)) for k, v in shared.items()}
    shared["emb"] = kge.astype(b16)
    shared["biascol"] = bias_mat

    in_maps = []
    for core in range(n_cores):
        lo = core * e_pc
        m = dict(shared)
        m["sidx"] = arrange(ei[0, lo:lo + e_pc].astype(np.int32))
        m["didx"] = arrange(ei[1, lo:lo + e_pc].astype(np.int32))
        in_maps.append(m)
    return in_maps


def _unshard(results, n_tiles=NTILES, n_cores=NCORES, e_pc=E_PC):
    # device returns logits [4, T*512]; softmax-select epilogue on host
    et = _EDGE_PERM["et"]
    ps = []
    for core in range(n_cores):
        lg = np.asarray(results[core]["out"], np.float32)[0:3, :e_pc]
        lg = lg - lg.max(axis=0, keepdims=True)
        ez = np.exp(lg)
        sel = np.take_along_axis(
            ez, et[core * e_pc:(core + 1) * e_pc][None, :], axis=0)[0]
        ps.append(sel / ez.sum(axis=0))
    cat = np.concatenate(ps)
    perm = _EDGE_PERM["perm"]
    if perm is not None:
        inv = np.empty_like(cat)
        inv[perm] = cat
        cat = inv
    return cat[:, None].astype(np.float32)


MMDT_MODE = "bf16"


def kernel(**inputs):
    from concourse.bass_utils import run_bass_kernel_spmd
    nc = _get_program(mmdt=MMDT_MODE)
    in_maps = _host_prep(inputs, mmdt=MMDT_MODE)
    res = run_bass_kernel_spmd(nc, in_maps, list(range(NCORES)))
    return _unshard(res.results)
